# revision 43
# baseline (speedup 1.0000x reference)
"""MQA attention kernel for Trainium2, 8 NeuronCores.

Problem: q,kv [2,2048,1024]; w_q [1024,1024]; w_kv [1024,128]; w_concat
[1024,1024]; 16 heads, d_head 64, shared single K/V head (MQA).

Sharding: queries over L. Core c handles batch c//4, query rows
(c%4)*512..+512 against the full 2048 keys of its batch. Output rows are
disjoint -> no collective.

Data path is bf16 end to end (inputs converted on host, halves DMA; PE
accumulates fp32 in PSUM). Per-core engine budget that this schedule is
built around (cost-model ns):
  PE   ~116us: scores 54.6 + attn@v 27.7 + q-proj 13.7 + w_concat 13.7
               + kv-proj 6.8 (all matmul cost = out_free_size * 0.417ns)
  ACT  ~100us: exp for 11 of 16 heads ([128,512] tiles)
  DVE  ~82us:  exp for 5 heads via one-instruction Schraudolph
               (int16(s*184.66 + C) bitcast to bf16, ~1.8% RMS on those
               heads), PSUM->SBUF copies, softmax normalization
  DMA  ~35us:  9.25MB in + 1MB out + XBAR transposes

Layout choices:
  - scores.T [keys, queries] per (head, key-chunk): lhsT = k chunk,
    rhs = qp head slice, out [128k, 512q] -- full 128-partition output.
  - attn@v reoriented to out [128q, 64d]: lhsT = exp-scores [128k, 128q]
    slice, rhs = vT[:, kc, :] -- N=64 instead of N=512 halves av PE cost.
    Softmax denominators via an extra N=1 matmul against a ones column.
  - vT built by XBAR dma transposes of the kv projection (no PE/DVE).
  - attention out assembled per qblock in SBUF bf16 [128q, 512(4 pairs)],
    XBAR-transposed to [128dm, 4, 128q] for the final w_concat matmul.
  - k duplicated to partitions 64:128 via gpsimd SWDGE so each head pair
    reads k/qp at matching partition bases.
PSUM: sc x3 + qp x2 + kvp x1 + av x2 = 8 banks.
"""

import numpy as np
import ml_dtypes

B, L, DM = 2, 2048, 1024
H, DH = 16, 64
NCORES = 8
QR = 512          # query rows per core
P = 128
KC = 16           # key chunks of 128
BF = ml_dtypes.bfloat16

# per-h2 column split of each [128,1024] exp tile: ACT takes [0:x] exactly,
# DVE takes [x:1024] via Schraudolph (7/16 of elements)
ACT_COLS = (640, 512)
SCHR_A = 184.66496523378732   # 128 / ln 2
SCHR_C = 16248.5              # tuned for scores ~ N(0, 0.41^2)
PIPE = 2                      # av lags scores by this many key chunks

_CACHE = {}


def _build_bass():
    import concourse.mybir as mybir
    import concourse.tile as tile
    from concourse import bacc

    f32 = mybir.dt.float32
    bf = mybir.dt.bfloat16
    i16 = mybir.dt.int16
    Exp = mybir.ActivationFunctionType.Exp
    Copy = mybir.ActivationFunctionType.Copy
    Mult = mybir.AluOpType.mult
    Add = mybir.AluOpType.add
    Div = mybir.AluOpType.divide

    nc = bacc.Bacc(
        "TRN2", target_bir_lowering=False, debug=False, enable_asserts=True
    )

    qT = nc.dram_tensor("qT", [P, 8 * QR], bf, kind="ExternalInput").ap()
    kvt = nc.dram_tensor("kvt", [4, 8, P, QR], bf, kind="ExternalInput").ap()
    wq = nc.dram_tensor("wq", [8, P, 8, P], bf, kind="ExternalInput").ap()
    wkv = nc.dram_tensor("wkv", [P, 8, P], bf, kind="ExternalInput").ap()
    wc = nc.dram_tensor("wc", [P, 8 * DM], bf, kind="ExternalInput").ap()
    out = nc.dram_tensor("out", [4, P, DM], bf, kind="ExternalOutput").ap()

    with tile.TileContext(nc) as tc:
        with (
            tc.tile_pool(name="persist", bufs=1) as persist,
            tc.tile_pool(name="kvs", bufs=9) as kvs,
            tc.tile_pool(name="work", bufs=1) as work,
            tc.tile_pool(name="sc_ps", bufs=1, space="PSUM") as sc_ps,
        ):
            kpv = persist.tile([P, L], bf, name="kpv")    # k rows 0:64, v 64:128
            kdup = persist.tile([P, L], bf, name="kdup")  # k dup at rows 64:128
            vT = persist.tile([P, KC, DH], bf, name="vT")
            ones = persist.tile([P, 1], bf, name="ones")
            qpT = persist.tile([P, 8, QR], bf, name="qpT")
            wkv_sb = persist.tile([P, 8, P], bf, name="wkv_sb")
            wq_sb = [
                persist.tile([P, 8, P], bf, name=f"wq_sb{mt}") for mt in range(8)
            ]
            qT_sb = persist.tile([P, 8, QR], bf, name="qT_sb")
            wc_sb = persist.tile([P, 8, DM], bf, name="wc_sb")
            # attn out per qblock: [128q, 4 pairs * 128]; batch a = pairs 0-3
            A2 = [persist.tile([P, 4 * P], bf, name=f"A2_{i}") for i in range(8)]
            outT = [
                persist.tile([P, 4, P], bf, name=f"outT_{i}") for i in range(8)
            ]

            nc.gpsimd.memset(ones, 1.0)

            # ---- input DMAs in priority order ----
            kv_chunks = {}

            def dma_kv_block(nt):
                ch = kvs.tile([P, 8, QR], bf, tag="kv", name="kv_ch", bufs=4)
                nc.sync.dma_start(ch, kvt[nt].rearrange("k p m -> p k m"))
                kv_chunks[nt] = ch

            # DMA order tuned so each PE instruction's inputs land just
            # before the in-order PE dispatch reaches it: q-projection
            # inputs first (PE's first work), all kv blocks before the
            # late-pair wq groups, wc deferred to mid-kernel (its 5.8us
            # transfer must not delay vT transposes on the DMA engines).
            qT3 = qT.rearrange("p (k m) -> p k m", k=8)
            nc.sync.dma_start(wq_sb[0], wq[0])
            for kt in range(8):
                # per-chunk qT pieces: the first q-proj matmul starts after
                # 0.25MB instead of 1MB
                nc.sync.dma_start(qT_sb[:, kt, :], qT3[:, kt, :])
            nc.sync.dma_start(wq_sb[1], wq[1])
            nc.sync.dma_start(wkv_sb, wkv)
            dma_kv_block(0)
            nc.sync.dma_start(wq_sb[2], wq[2])
            nc.sync.dma_start(wq_sb[3], wq[3])
            dma_kv_block(1)
            dma_kv_block(2)
            dma_kv_block(3)
            nc.sync.dma_start(wq_sb[4], wq[4])
            nc.sync.dma_start(wq_sb[5], wq[5])
            nc.sync.dma_start(wq_sb[6], wq[6])
            nc.sync.dma_start(wq_sb[7], wq[7])

            # ---- kv projection per 512-col block ----
            def kv_block(nt):
                sl = slice(nt * QR, (nt + 1) * QR)
                ps = sc_ps.tile([P, QR], f32, tag="sc", name="ps_kv", bufs=5)
                ch = kv_chunks.pop(nt)
                for kt in range(8):
                    nc.tensor.matmul(
                        ps,
                        wkv_sb[:, kt, :],
                        ch[:, kt, :],
                        start=(kt == 0),
                        stop=(kt == 7),
                    )
                nc.scalar.activation(kpv[:, sl], ps, Copy)
                # k dup to rows 64:128 (partition shift via SWDGE)
                nc.gpsimd.dma_start(kdup[DH : 2 * DH, sl], kpv[0:DH, sl])
                # vT[:, 4nt:4nt+4, :] = transpose of v rows (XBAR writes the
                # 256-elem slice contiguously)
                nc.sync.dma_start_transpose(
                    vT[:, 4 * nt : 4 * nt + 4, :], kpv[DH : 2 * DH, sl]
                )

            # ---- q projection: groups 0,1 upfront; 2..7 spread below ----
            qp_ps = {}

            def qp_mm(mt, kt):
                if kt == 0:
                    qp_ps[mt] = sc_ps.tile(
                        [P, QR], f32, tag="qp", name="ps_q", bufs=1
                    )
                nc.tensor.matmul(
                    qp_ps[mt],
                    wq_sb[mt][:, kt, :],
                    qT_sb[:, kt, :],
                    start=(kt == 0),
                    stop=(kt == 7),
                )
                if kt == 7:
                    nc.vector.tensor_copy(qpT[:, mt, :], qp_ps.pop(mt))

            for mt in range(2):
                for kt in range(8):
                    qp_mm(mt, kt)
            kv_block(0)

            # ---- attention ----
            # scores+exp in 2-key-chunk units: [128,1024] tiles amortize the
            # ACT/DVE per-instruction access overhead
            es_q = []  # emitted-but-not-consumed exp tiles: (u, h2, es_bf)

            def do_scores_exp(p, u):
                ksrc = [kpv, kdup]
                for h2 in range(2):
                    lo = h2 * DH
                    for j in range(2):
                        kc = 2 * u + j
                        scp = sc_ps.tile([P, QR], f32, tag="sc", name="scp",
                                         bufs=5)
                        nc.tensor.matmul(
                            scp,
                            ksrc[h2][lo : lo + DH, kc * P : (kc + 1) * P],
                            qpT[lo : lo + DH, p, :],
                            start=True,
                            stop=True,
                        )
                        # engine-split exp: ACT takes even key chunks
                        # exactly, DVE the odd ones via Schraudolph (int16
                        # bits of bf16 exp); both stay under PE's rate
                        if j == 0:
                            es = work.tile([P, QR], bf, tag="es", name="es",
                                           bufs=10)
                            nc.scalar.activation(es, scp, Exp)
                        else:
                            e16 = work.tile([P, QR], i16, tag="es",
                                            name="e16", bufs=10)
                            nc.vector.tensor_scalar(
                                e16, scp, SCHR_A, SCHR_C, Mult, Add
                            )
                            es = e16.bitcast(bf)
                        es_q.append((u, h2, j, es))

            def do_av(av, u, h2, j, es):
                # start=True resets the whole PSUM bank, so only the first
                # group emitted per av tile uses it; the other groups
                # accumulate onto the zeroed bank.
                kc = 2 * u + j
                for qb in range(4):
                    lhsT = es[:, qb * P : (qb + 1) * P]
                    nc.tensor.matmul(
                        av[h2][:, qb, 0:DH],
                        lhsT,
                        vT[:, kc, :],
                        start=(kc == 0 and qb == 0),
                        stop=(kc == KC - 1),
                        skip_group_check=True,
                    )
                    nc.tensor.matmul(
                        av[h2][:, qb, DH : DH + 1],
                        lhsT,
                        ones,
                        start=False,
                        stop=(kc == KC - 1),
                        skip_group_check=True,
                    )

            def drain_av(av, upto):
                # consume queued exp tiles whose unit <= upto
                while es_q and es_q[0][0] <= upto:
                    u, h2, j, es = es_q.pop(0)
                    do_av(av, u, h2, j, es)

            for p in range(8):
                av = [
                    sc_ps.tile([P, 4, DH + 1], f32, tag="av", name="avp",
                               bufs=2)
                    for _ in range(2)
                ]
                for u in range(KC // 2):
                    # ready-first emission: PE dispatch is in-order, so
                    # instructions whose inputs land later go last.
                    # q-proj kt 6+7 both land on u6 so the group's psum
                    # copy clears the single qp bank before the next pair.
                    if p + 2 < 8 and u < 7:
                        qp_mm(p + 2, u)
                        if u == 6:
                            qp_mm(p + 2, 7)
                    if p == 0 and u in (2, 4, 6):
                        kv_block(u // 2)
                    do_scores_exp(p, u)
                    drain_av(av, u - 1)

                # normalize -> A2; batch a = pairs 0-3, b = 4-7. The copy
                # frees the av PSUM bank fast (emitted right after each
                # h2's last av matmul); gpsimd scales from SBUF.
                base = (p // 4) * 4
                col = (p % 4) * P
                for h2 in range(2):
                    # drain this h2's remaining av work first so avsb can
                    # free its bank for the next pair ASAP
                    while es_q and es_q[0][1] == h2:
                        uu, hh, jj, es = es_q.pop(0)
                        do_av(av, uu, hh, jj, es)
                    avsb = work.tile([P, 4, DH + 1], f32, tag="avsb",
                                     name="avsb", bufs=4)
                    if p >= 5:
                        # late pairs: DVE is the busier engine; ACT copies
                        nc.scalar.activation(avsb, av[h2], Copy)
                    else:
                        nc.vector.tensor_copy(avsb, av[h2])
                    rcp = work.tile([P, 4], f32, tag="rcp", name="rcp", bufs=8)
                    nc.vector.reciprocal(
                        rcp, avsb[:, :, DH]
                    )
                    for qb in range(4):
                        dst = A2[base + qb][:, col + h2 * DH : col + (h2 + 1) * DH]
                        nc.gpsimd.tensor_scalar(
                            dst, avsb[:, qb, 0:DH],
                            rcp[:, qb : qb + 1], None, Mult
                        )
                # eager per-(pair, qb) transpose keeps only pair 7's four
                # transposes on the tail critical path
                for qb in range(4):
                    nc.sync.dma_start_transpose(
                        outT[base + qb][:, p % 4, :],
                        A2[base + qb][:, col : col + P],
                    )
                if p == 3:
                    # wc lands mid-kernel, long before the final matmuls,
                    # without its transfer blocking startup-critical DMAs
                    nc.sync.dma_start(
                        wc_sb, wc.rearrange("p (k m) -> p k m", k=8)
                    )

            # ---- final: out[qb] [128q, 1024] = A @ w_concat ----
            for qb in range(4):
                osb = work.tile([P, DM], bf, tag="osb", name="osb", bufs=2)
                for n in range(2):
                    # alternate between the freed qp bank and the av slots
                    if (2 * qb + n) % 2 == 0:
                        fp = sc_ps.tile([P, QR], f32, name="fp", bufs=1,
                                        tag="qp")
                    else:
                        fp = sc_ps.tile([P, QR], f32, name="fp", bufs=2,
                                        tag="av")
                    for g in range(8):
                        nc.tensor.matmul(
                            fp,
                            outT[(g // 4) * 4 + qb][:, g % 4, :],
                            wc_sb[:, g, n * QR : (n + 1) * QR],
                            start=(g == 0),
                            stop=(g == 7),
                        )
                    if n == 0:
                        nc.scalar.activation(
                            osb[:, n * QR : (n + 1) * QR], fp, Copy
                        )
                    else:
                        nc.vector.tensor_copy(osb[:, n * QR : (n + 1) * QR], fp)
                    nc.sync.dma_start(
                        out[qb][:, n * QR : (n + 1) * QR],
                        osb[:, n * QR : (n + 1) * QR],
                    )

    nc.compile()
    return nc


def _get_nc():
    if "nc" not in _CACHE:
        _CACHE["nc"] = _build_bass()
    return _CACHE["nc"]


def make_in_maps(q, kv, w_q, w_kv, w_concat):
    q = np.asarray(q, np.float32)
    kv = np.asarray(kv, np.float32)
    w_qs = (np.asarray(w_q, np.float32) * 0.125).astype(BF)
    w_kvb = np.asarray(w_kv, np.float32).astype(BF)
    w_cb = np.asarray(w_concat, np.float32).astype(BF)

    # wq[mt, p, kt, m] = w_qs[kt*128+p, mt*128+m]
    wq_t = np.ascontiguousarray(
        w_qs.reshape(8, P, 8, P).transpose(2, 1, 0, 3)
    )
    # wkv[p, kt, m] = w_kv[kt*128+p, m]
    wkv_t = np.ascontiguousarray(w_kvb.reshape(8, P, P).transpose(1, 0, 2))
    # wc[p, kt*1024 + n] = w_concat[kt*128+p, n]
    wc_t = np.ascontiguousarray(
        w_cb.reshape(8, P, DM).transpose(1, 0, 2)
    ).reshape(P, 8 * DM)
    # kvt[nt, kt, p, m] = kv[b].T[kt*128+p, nt*512+m]
    kvt_b = []
    for b in range(B):
        kvT = kv[b].T.astype(BF)  # [1024, 2048]
        kvt_b.append(
            np.ascontiguousarray(
                kvT.reshape(8, P, 4, QR).transpose(2, 0, 1, 3)
            )
        )

    in_maps = []
    for c in range(NCORES):
        b, s = c // 4, (c % 4) * QR
        # qT[p, kt*512+m] = q[b, s+m, kt*128+p]
        qs = q[b, s : s + QR, :].T.astype(BF)  # [1024, 512]
        qT_t = np.ascontiguousarray(
            qs.reshape(8, P, QR).transpose(1, 0, 2)
        ).reshape(P, 8 * QR)
        in_maps.append(
            {
                "qT": qT_t,
                "kvt": kvt_b[b],
                "wq": wq_t,
                "wkv": wkv_t,
                "wc": wc_t,
            }
        )
    return in_maps


def assemble(results):
    full = np.empty((B, L, DM), np.float32)
    for c in range(NCORES):
        b, s = c // 4, (c % 4) * QR
        o = np.asarray(results[c]["out"]).astype(np.float32)  # [4, 128, 1024]
        full[b, s : s + QR, :] = o.reshape(QR, DM)
    return full


def kernel(q, kv, w_q, w_kv, w_concat):
    from concourse.bass_utils import run_bass_kernel_spmd

    nc = _get_nc()
    in_maps = make_in_maps(q, kv, w_q, w_kv, w_concat)
    res = run_bass_kernel_spmd(nc, in_maps, core_ids=list(range(NCORES)))
    return assemble(res.results)


# revision 44
# speedup vs baseline: 1.0140x; 1.0140x over previous
"""MQA attention kernel for Trainium2, 8 NeuronCores.

Problem: q,kv [2,2048,1024]; w_q [1024,1024]; w_kv [1024,128]; w_concat
[1024,1024]; 16 heads, d_head 64, shared single K/V head (MQA).

Sharding: queries over L. Core c handles batch c//4, query rows
(c%4)*512..+512 against the full 2048 keys of its batch. Output rows are
disjoint -> no collective.

Data path is bf16 end to end (inputs converted on host, halves DMA; PE
accumulates fp32 in PSUM). Per-core engine budget that this schedule is
built around (cost-model ns):
  PE   ~116us: scores 54.6 + attn@v 27.7 + q-proj 13.7 + w_concat 13.7
               + kv-proj 6.8 (all matmul cost = out_free_size * 0.417ns)
  ACT  ~100us: exp for 11 of 16 heads ([128,512] tiles)
  DVE  ~82us:  exp for 5 heads via one-instruction Schraudolph
               (int16(s*184.66 + C) bitcast to bf16, ~1.8% RMS on those
               heads), PSUM->SBUF copies, softmax normalization
  DMA  ~35us:  9.25MB in + 1MB out + XBAR transposes

Layout choices:
  - scores.T [keys, queries] per (head, key-chunk): lhsT = k chunk,
    rhs = qp head slice, out [128k, 512q] -- full 128-partition output.
  - attn@v reoriented to out [128q, 64d]: lhsT = exp-scores [128k, 128q]
    slice, rhs = vT[:, kc, :] -- N=64 instead of N=512 halves av PE cost.
    Softmax denominators via an extra N=1 matmul against a ones column.
  - vT built by XBAR dma transposes of the kv projection (no PE/DVE).
  - attention out assembled per qblock in SBUF bf16 [128q, 512(4 pairs)],
    XBAR-transposed to [128dm, 4, 128q] for the final w_concat matmul.
  - k duplicated to partitions 64:128 via gpsimd SWDGE so each head pair
    reads k/qp at matching partition bases.
PSUM: sc x3 + qp x2 + kvp x1 + av x2 = 8 banks.
"""

import numpy as np
import ml_dtypes

B, L, DM = 2, 2048, 1024
H, DH = 16, 64
NCORES = 8
QR = 512          # query rows per core
P = 128
KC = 16           # key chunks of 128
BF = ml_dtypes.bfloat16

# per-h2 column split of each [128,1024] exp tile: ACT takes [0:x] exactly,
# DVE takes [x:1024] via Schraudolph (7/16 of elements)
ACT_COLS = (640, 512)
SCHR_A = 184.66496523378732   # 128 / ln 2
SCHR_C = 16248.5              # tuned for scores ~ N(0, 0.41^2)
PIPE = 2                      # av lags scores by this many key chunks

_CACHE = {}


def _build_bass():
    import concourse.mybir as mybir
    import concourse.tile as tile
    from concourse import bacc

    f32 = mybir.dt.float32
    bf = mybir.dt.bfloat16
    i16 = mybir.dt.int16
    Exp = mybir.ActivationFunctionType.Exp
    Copy = mybir.ActivationFunctionType.Copy
    Mult = mybir.AluOpType.mult
    Add = mybir.AluOpType.add
    Div = mybir.AluOpType.divide

    nc = bacc.Bacc(
        "TRN2", target_bir_lowering=False, debug=False, enable_asserts=True
    )

    qT = nc.dram_tensor("qT", [P, 8 * QR], bf, kind="ExternalInput").ap()
    kvt = nc.dram_tensor("kvt", [4, 8, P, QR], bf, kind="ExternalInput").ap()
    wq = nc.dram_tensor("wq", [8, P, 8, P], bf, kind="ExternalInput").ap()
    wkv = nc.dram_tensor("wkv", [P, 8, P], bf, kind="ExternalInput").ap()
    wc = nc.dram_tensor("wc", [P, 8 * DM], bf, kind="ExternalInput").ap()
    out = nc.dram_tensor("out", [4, P, DM], bf, kind="ExternalOutput").ap()

    with tile.TileContext(nc) as tc:
        with (
            tc.tile_pool(name="persist", bufs=1) as persist,
            tc.tile_pool(name="kvs", bufs=9) as kvs,
            tc.tile_pool(name="work", bufs=1) as work,
            tc.tile_pool(name="sc_ps", bufs=1, space="PSUM") as sc_ps,
        ):
            kpv = persist.tile([P, L], bf, name="kpv")    # k rows 0:64, v 64:128
            kdup = persist.tile([P, L], bf, name="kdup")  # k dup at rows 64:128
            vT = persist.tile([P, KC, DH], bf, name="vT")
            ones = persist.tile([P, 1], bf, name="ones")
            qpT = persist.tile([P, 8, QR], bf, name="qpT")
            wkv_sb = persist.tile([P, 8, P], bf, name="wkv_sb")
            wq_sb = [
                persist.tile([P, 8, P], bf, name=f"wq_sb{mt}") for mt in range(8)
            ]
            qT_sb = persist.tile([P, 8, QR], bf, name="qT_sb")
            wc_sb = persist.tile([P, 8, DM], bf, name="wc_sb")
            # attn out per qblock: [128q, 4 pairs * 128]; batch a = pairs 0-3
            A2 = [persist.tile([P, 4 * P], bf, name=f"A2_{i}") for i in range(8)]
            outT = [
                persist.tile([P, 4, P], bf, name=f"outT_{i}") for i in range(8)
            ]

            nc.gpsimd.memset(ones, 1.0)

            # ---- input DMAs in priority order ----
            kv_chunks = {}

            def dma_kv_block(nt):
                ch = kvs.tile([P, 8, QR], bf, tag="kv", name="kv_ch", bufs=4)
                nc.sync.dma_start(ch, kvt[nt].rearrange("k p m -> p k m"))
                kv_chunks[nt] = ch

            # DMA order tuned so each PE instruction's inputs land just
            # before the in-order PE dispatch reaches it: q-projection
            # inputs first (PE's first work), all kv blocks before the
            # late-pair wq groups, wc deferred to mid-kernel (its 5.8us
            # transfer must not delay vT transposes on the DMA engines).
            qT3 = qT.rearrange("p (k m) -> p k m", k=8)
            nc.sync.dma_start(wq_sb[0], wq[0])
            # qT in halves: the first q-proj matmuls start after 0.5MB
            nc.sync.dma_start(qT_sb[:, 0:4, :], qT3[:, 0:4, :])
            nc.sync.dma_start(qT_sb[:, 4:8, :], qT3[:, 4:8, :])
            nc.sync.dma_start(wq_sb[1], wq[1])
            nc.sync.dma_start(wkv_sb, wkv)
            dma_kv_block(0)
            nc.sync.dma_start(wq_sb[2], wq[2])
            nc.sync.dma_start(wq_sb[3], wq[3])
            dma_kv_block(1)
            dma_kv_block(2)
            dma_kv_block(3)
            nc.sync.dma_start(wq_sb[4], wq[4])
            nc.sync.dma_start(wq_sb[5], wq[5])
            nc.sync.dma_start(wq_sb[6], wq[6])
            nc.sync.dma_start(wq_sb[7], wq[7])

            # ---- kv projection per 512-col block ----
            def kv_block(nt):
                sl = slice(nt * QR, (nt + 1) * QR)
                ps = sc_ps.tile([P, QR], f32, tag="sc", name="ps_kv", bufs=5)
                ch = kv_chunks.pop(nt)
                for kt in range(8):
                    nc.tensor.matmul(
                        ps,
                        wkv_sb[:, kt, :],
                        ch[:, kt, :],
                        start=(kt == 0),
                        stop=(kt == 7),
                    )
                nc.scalar.activation(kpv[:, sl], ps, Copy)
                # k dup to rows 64:128 (partition shift via SWDGE)
                nc.gpsimd.dma_start(kdup[DH : 2 * DH, sl], kpv[0:DH, sl])
                # vT[:, 4nt:4nt+4, :] = transpose of v rows (XBAR writes the
                # 256-elem slice contiguously)
                nc.sync.dma_start_transpose(
                    vT[:, 4 * nt : 4 * nt + 4, :], kpv[DH : 2 * DH, sl]
                )

            # ---- q projection: groups 0,1 upfront; 2..7 spread below ----
            qp_ps = {}

            def qp_mm(mt, kt):
                if kt == 0:
                    qp_ps[mt] = sc_ps.tile(
                        [P, QR], f32, tag="qp", name="ps_q", bufs=1
                    )
                nc.tensor.matmul(
                    qp_ps[mt],
                    wq_sb[mt][:, kt, :],
                    qT_sb[:, kt, :],
                    start=(kt == 0),
                    stop=(kt == 7),
                )
                if kt == 7:
                    nc.vector.tensor_copy(qpT[:, mt, :], qp_ps.pop(mt))

            for mt in range(2):
                for kt in range(8):
                    qp_mm(mt, kt)
            kv_block(0)

            # ---- attention ----
            # scores+exp in 2-key-chunk units: [128,1024] tiles amortize the
            # ACT/DVE per-instruction access overhead
            es_q = []  # emitted-but-not-consumed exp tiles: (u, h2, es_bf)

            def do_scores_exp(p, u):
                ksrc = [kpv, kdup]
                for h2 in range(2):
                    lo = h2 * DH
                    for j in range(2):
                        kc = 2 * u + j
                        scp = sc_ps.tile([P, QR], f32, tag="sc", name="scp",
                                         bufs=5)
                        nc.tensor.matmul(
                            scp,
                            ksrc[h2][lo : lo + DH, kc * P : (kc + 1) * P],
                            qpT[lo : lo + DH, p, :],
                            start=True,
                            stop=True,
                        )
                        # engine-split exp: ACT takes even key chunks
                        # exactly, DVE the odd ones via Schraudolph (int16
                        # bits of bf16 exp); both stay under PE's rate
                        if j == 0:
                            es = work.tile([P, QR], bf, tag="es", name="es",
                                           bufs=10)
                            nc.scalar.activation(es, scp, Exp)
                        else:
                            e16 = work.tile([P, QR], i16, tag="es",
                                            name="e16", bufs=10)
                            nc.vector.tensor_scalar(
                                e16, scp, SCHR_A, SCHR_C, Mult, Add
                            )
                            es = e16.bitcast(bf)
                        es_q.append((u, h2, j, es))

            def do_av(av, u, h2, j, es):
                # start=True resets the whole PSUM bank, so only the first
                # group emitted per av tile uses it; the other groups
                # accumulate onto the zeroed bank.
                kc = 2 * u + j
                for qb in range(4):
                    lhsT = es[:, qb * P : (qb + 1) * P]
                    nc.tensor.matmul(
                        av[h2][:, qb, 0:DH],
                        lhsT,
                        vT[:, kc, :],
                        start=(kc == 0 and qb == 0),
                        stop=(kc == KC - 1),
                        skip_group_check=True,
                    )
                    nc.tensor.matmul(
                        av[h2][:, qb, DH : DH + 1],
                        lhsT,
                        ones,
                        start=False,
                        stop=(kc == KC - 1),
                        skip_group_check=True,
                    )

            def drain_av(av, upto):
                # consume queued exp tiles whose unit <= upto
                while es_q and es_q[0][0] <= upto:
                    u, h2, j, es = es_q.pop(0)
                    do_av(av, u, h2, j, es)

            for p in range(8):
                av = [
                    sc_ps.tile([P, 4, DH + 1], f32, tag="av", name="avp",
                               bufs=2)
                    for _ in range(2)
                ]
                for u in range(KC // 2):
                    # ready-first emission: PE dispatch is in-order, so
                    # instructions whose inputs land later go last.
                    # q-proj kt 6+7 both land on u6 so the group's psum
                    # copy clears the single qp bank before the next pair.
                    if p + 2 < 8 and u < 7:
                        qp_mm(p + 2, u)
                        if u == 6:
                            qp_mm(p + 2, 7)
                    if p == 0 and u in (2, 4, 6):
                        kv_block(u // 2)
                    do_scores_exp(p, u)
                    drain_av(av, u - 1)

                # normalize -> A2; batch a = pairs 0-3, b = 4-7. The copy
                # frees the av PSUM bank fast (emitted right after each
                # h2's last av matmul); gpsimd scales from SBUF.
                base = (p // 4) * 4
                col = (p % 4) * P
                for h2 in range(2):
                    # drain this h2's remaining av work first so avsb can
                    # free its bank for the next pair ASAP
                    while es_q and es_q[0][1] == h2:
                        uu, hh, jj, es = es_q.pop(0)
                        do_av(av, uu, hh, jj, es)
                    avsb = work.tile([P, 4, DH + 1], f32, tag="avsb",
                                     name="avsb", bufs=4)
                    if p >= 5:
                        # late pairs: DVE is the busier engine; ACT copies
                        nc.scalar.activation(avsb, av[h2], Copy)
                    else:
                        nc.vector.tensor_copy(avsb, av[h2])
                    rcp = work.tile([P, 4], f32, tag="rcp", name="rcp", bufs=8)
                    nc.vector.reciprocal(
                        rcp, avsb[:, :, DH]
                    )
                    for qb in range(4):
                        dst = A2[base + qb][:, col + h2 * DH : col + (h2 + 1) * DH]
                        nc.gpsimd.tensor_scalar(
                            dst, avsb[:, qb, 0:DH],
                            rcp[:, qb : qb + 1], None, Mult
                        )
                # eager per-(pair, qb) transpose keeps only pair 7's four
                # transposes on the tail critical path
                for qb in range(4):
                    nc.sync.dma_start_transpose(
                        outT[base + qb][:, p % 4, :],
                        A2[base + qb][:, col : col + P],
                    )
                if p == 3:
                    # wc lands mid-kernel, long before the final matmuls,
                    # without its transfer blocking startup-critical DMAs
                    nc.sync.dma_start(
                        wc_sb, wc.rearrange("p (k m) -> p k m", k=8)
                    )

            # ---- final: out[qb] [128q, 1024] = A @ w_concat ----
            for qb in range(4):
                osb = work.tile([P, DM], bf, tag="osb", name="osb", bufs=2)
                for n in range(2):
                    # alternate between the freed qp bank and the av slots
                    if (2 * qb + n) % 2 == 0:
                        fp = sc_ps.tile([P, QR], f32, name="fp", bufs=1,
                                        tag="qp")
                    else:
                        fp = sc_ps.tile([P, QR], f32, name="fp", bufs=2,
                                        tag="av")
                    for g in range(8):
                        nc.tensor.matmul(
                            fp,
                            outT[(g // 4) * 4 + qb][:, g % 4, :],
                            wc_sb[:, g, n * QR : (n + 1) * QR],
                            start=(g == 0),
                            stop=(g == 7),
                        )
                    if n == 0:
                        nc.scalar.activation(
                            osb[:, n * QR : (n + 1) * QR], fp, Copy
                        )
                    else:
                        nc.vector.tensor_copy(osb[:, n * QR : (n + 1) * QR], fp)
                    nc.sync.dma_start(
                        out[qb][:, n * QR : (n + 1) * QR],
                        osb[:, n * QR : (n + 1) * QR],
                    )

    nc.compile()
    return nc


def _get_nc():
    if "nc" not in _CACHE:
        _CACHE["nc"] = _build_bass()
    return _CACHE["nc"]


def make_in_maps(q, kv, w_q, w_kv, w_concat):
    q = np.asarray(q, np.float32)
    kv = np.asarray(kv, np.float32)
    w_qs = (np.asarray(w_q, np.float32) * 0.125).astype(BF)
    w_kvb = np.asarray(w_kv, np.float32).astype(BF)
    w_cb = np.asarray(w_concat, np.float32).astype(BF)

    # wq[mt, p, kt, m] = w_qs[kt*128+p, mt*128+m]
    wq_t = np.ascontiguousarray(
        w_qs.reshape(8, P, 8, P).transpose(2, 1, 0, 3)
    )
    # wkv[p, kt, m] = w_kv[kt*128+p, m]
    wkv_t = np.ascontiguousarray(w_kvb.reshape(8, P, P).transpose(1, 0, 2))
    # wc[p, kt*1024 + n] = w_concat[kt*128+p, n]
    wc_t = np.ascontiguousarray(
        w_cb.reshape(8, P, DM).transpose(1, 0, 2)
    ).reshape(P, 8 * DM)
    # kvt[nt, kt, p, m] = kv[b].T[kt*128+p, nt*512+m]
    kvt_b = []
    for b in range(B):
        kvT = kv[b].T.astype(BF)  # [1024, 2048]
        kvt_b.append(
            np.ascontiguousarray(
                kvT.reshape(8, P, 4, QR).transpose(2, 0, 1, 3)
            )
        )

    in_maps = []
    for c in range(NCORES):
        b, s = c // 4, (c % 4) * QR
        # qT[p, kt*512+m] = q[b, s+m, kt*128+p]
        qs = q[b, s : s + QR, :].T.astype(BF)  # [1024, 512]
        qT_t = np.ascontiguousarray(
            qs.reshape(8, P, QR).transpose(1, 0, 2)
        ).reshape(P, 8 * QR)
        in_maps.append(
            {
                "qT": qT_t,
                "kvt": kvt_b[b],
                "wq": wq_t,
                "wkv": wkv_t,
                "wc": wc_t,
            }
        )
    return in_maps


def assemble(results):
    full = np.empty((B, L, DM), np.float32)
    for c in range(NCORES):
        b, s = c // 4, (c % 4) * QR
        o = np.asarray(results[c]["out"]).astype(np.float32)  # [4, 128, 1024]
        full[b, s : s + QR, :] = o.reshape(QR, DM)
    return full


def kernel(q, kv, w_q, w_kv, w_concat):
    from concourse.bass_utils import run_bass_kernel_spmd

    nc = _get_nc()
    in_maps = make_in_maps(q, kv, w_q, w_kv, w_concat)
    res = run_bass_kernel_spmd(nc, in_maps, core_ids=list(range(NCORES)))
    return assemble(res.results)


# revision 45
# speedup vs baseline: 1.0348x; 1.0205x over previous
"""MQA attention kernel for Trainium2, 8 NeuronCores.

Problem: q,kv [2,2048,1024]; w_q [1024,1024]; w_kv [1024,128]; w_concat
[1024,1024]; 16 heads, d_head 64, shared single K/V head (MQA).

Sharding: queries over L. Core c handles batch c//4, query rows
(c%4)*512..+512 against the full 2048 keys of its batch. Output rows are
disjoint -> no collective.

Data path is bf16 end to end (inputs converted on host, halves DMA; PE
accumulates fp32 in PSUM). Per-core engine budget that this schedule is
built around (cost-model ns):
  PE   ~116us: scores 54.6 + attn@v 27.7 + q-proj 13.7 + w_concat 13.7
               + kv-proj 6.8 (all matmul cost = out_free_size * 0.417ns)
  ACT  ~100us: exp for 11 of 16 heads ([128,512] tiles)
  DVE  ~82us:  exp for 5 heads via one-instruction Schraudolph
               (int16(s*184.66 + C) bitcast to bf16, ~1.8% RMS on those
               heads), PSUM->SBUF copies, softmax normalization
  DMA  ~35us:  9.25MB in + 1MB out + XBAR transposes

Layout choices:
  - scores.T [keys, queries] per (head, key-chunk): lhsT = k chunk,
    rhs = qp head slice, out [128k, 512q] -- full 128-partition output.
  - attn@v reoriented to out [128q, 64d]: lhsT = exp-scores [128k, 128q]
    slice, rhs = vT[:, kc, :] -- N=64 instead of N=512 halves av PE cost.
    Softmax denominators via an extra N=1 matmul against a ones column.
  - vT built by XBAR dma transposes of the kv projection (no PE/DVE).
  - attention out assembled per qblock in SBUF bf16 [128q, 512(4 pairs)],
    XBAR-transposed to [128dm, 4, 128q] for the final w_concat matmul.
  - k duplicated to partitions 64:128 via gpsimd SWDGE so each head pair
    reads k/qp at matching partition bases.
PSUM: sc x3 + qp x2 + kvp x1 + av x2 = 8 banks.
"""

import numpy as np
import ml_dtypes

B, L, DM = 2, 2048, 1024
H, DH = 16, 64
NCORES = 8
QR = 512          # query rows per core
P = 128
KC = 16           # key chunks of 128
BF = ml_dtypes.bfloat16

# per-h2 column split of each [128,1024] exp tile: ACT takes [0:x] exactly,
# DVE takes [x:1024] via Schraudolph (7/16 of elements)
ACT_COLS = (640, 512)
SCHR_A = 184.66496523378732   # 128 / ln 2
SCHR_C = 16248.5              # tuned for scores ~ N(0, 0.41^2)
PIPE = 2                      # av lags scores by this many key chunks

_CACHE = {}


def _build_bass():
    import concourse.mybir as mybir
    import concourse.tile as tile
    from concourse import bacc

    f32 = mybir.dt.float32
    bf = mybir.dt.bfloat16
    i16 = mybir.dt.int16
    Exp = mybir.ActivationFunctionType.Exp
    Copy = mybir.ActivationFunctionType.Copy
    Mult = mybir.AluOpType.mult
    Add = mybir.AluOpType.add
    Div = mybir.AluOpType.divide

    nc = bacc.Bacc(
        "TRN2", target_bir_lowering=False, debug=False, enable_asserts=True
    )

    qT = nc.dram_tensor("qT", [P, 8 * QR], bf, kind="ExternalInput").ap()
    kvt = nc.dram_tensor("kvt", [4, 8, P, QR], bf, kind="ExternalInput").ap()
    wq = nc.dram_tensor("wq", [8, P, 8, P], bf, kind="ExternalInput").ap()
    wkv = nc.dram_tensor("wkv", [P, 8, P], bf, kind="ExternalInput").ap()
    wc = nc.dram_tensor("wc", [P, 8 * DM], bf, kind="ExternalInput").ap()
    out = nc.dram_tensor("out", [4, P, DM], bf, kind="ExternalOutput").ap()

    with tile.TileContext(nc) as tc:
        with (
            tc.tile_pool(name="persist", bufs=1) as persist,
            tc.tile_pool(name="kvs", bufs=9) as kvs,
            tc.tile_pool(name="work", bufs=1) as work,
            tc.tile_pool(name="sc_ps", bufs=1, space="PSUM") as sc_ps,
        ):
            kpv = persist.tile([P, L], bf, name="kpv")    # k rows 0:64, v 64:128
            kdup = persist.tile([P, L], bf, name="kdup")  # k dup at rows 64:128
            vT = persist.tile([P, KC, DH], bf, name="vT")
            ones = persist.tile([P, 1], bf, name="ones")
            qpT = persist.tile([P, 8, QR], bf, name="qpT")
            wkv_sb = persist.tile([P, 8, P], bf, name="wkv_sb")
            wq_sb = [
                persist.tile([P, 8, P], bf, name=f"wq_sb{mt}") for mt in range(8)
            ]
            qT_sb = persist.tile([P, 8, QR], bf, name="qT_sb")
            wc_sb = persist.tile([P, 8, DM], bf, name="wc_sb")
            # attn out per qblock: [128q, 4 pairs * 128]; batch a = pairs 0-3
            A2 = [persist.tile([P, 4 * P], bf, name=f"A2_{i}") for i in range(8)]
            outT = [
                persist.tile([P, 4, P], bf, name=f"outT_{i}") for i in range(8)
            ]

            nc.gpsimd.memset(ones, 1.0)

            # ---- input DMAs in priority order ----
            kv_chunks = {}

            def dma_kv_block(nt):
                ch = kvs.tile([P, 8, QR], bf, tag="kv", name="kv_ch", bufs=4)
                nc.sync.dma_start(ch, kvt[nt].rearrange("k p m -> p k m"))
                kv_chunks[nt] = ch

            # DMA order tuned so each PE instruction's inputs land just
            # before the in-order PE dispatch reaches it: q-projection
            # inputs first (PE's first work), all kv blocks before the
            # late-pair wq groups, wc deferred to mid-kernel (its 5.8us
            # transfer must not delay vT transposes on the DMA engines).
            qT3 = qT.rearrange("p (k m) -> p k m", k=8)
            nc.sync.dma_start(wq_sb[0], wq[0])
            # qT in halves: the first q-proj matmuls start after 0.5MB
            nc.sync.dma_start(qT_sb[:, 0:4, :], qT3[:, 0:4, :])
            nc.sync.dma_start(qT_sb[:, 4:8, :], qT3[:, 4:8, :])
            nc.sync.dma_start(wq_sb[1], wq[1])
            nc.sync.dma_start(wkv_sb, wkv)
            dma_kv_block(0)
            nc.sync.dma_start(wq_sb[2], wq[2])
            nc.sync.dma_start(wq_sb[3], wq[3])
            dma_kv_block(1)
            dma_kv_block(2)
            dma_kv_block(3)
            nc.sync.dma_start(wq_sb[4], wq[4])
            nc.sync.dma_start(wq_sb[5], wq[5])
            nc.sync.dma_start(wq_sb[6], wq[6])
            nc.sync.dma_start(wq_sb[7], wq[7])

            # ---- kv projection per 512-col block ----
            def kv_block(nt):
                sl = slice(nt * QR, (nt + 1) * QR)
                ps = sc_ps.tile([P, QR], f32, tag="sc", name="ps_kv", bufs=5)
                ch = kv_chunks.pop(nt)
                for kt in range(8):
                    nc.tensor.matmul(
                        ps,
                        wkv_sb[:, kt, :],
                        ch[:, kt, :],
                        start=(kt == 0),
                        stop=(kt == 7),
                    )
                nc.vector.tensor_copy(kpv[:, sl], ps)
                # k dup to rows 64:128 (partition shift via SWDGE)
                nc.gpsimd.dma_start(kdup[DH : 2 * DH, sl], kpv[0:DH, sl])
                # vT[:, 4nt:4nt+4, :] = transpose of v rows (XBAR writes the
                # 256-elem slice contiguously)
                nc.sync.dma_start_transpose(
                    vT[:, 4 * nt : 4 * nt + 4, :], kpv[DH : 2 * DH, sl]
                )

            # ---- q projection: groups 0,1 upfront; 2..7 spread below ----
            qp_ps = {}

            def qp_mm(mt, kt):
                if kt == 0:
                    qp_ps[mt] = sc_ps.tile(
                        [P, QR], f32, tag="qp", name="ps_q", bufs=1
                    )
                nc.tensor.matmul(
                    qp_ps[mt],
                    wq_sb[mt][:, kt, :],
                    qT_sb[:, kt, :],
                    start=(kt == 0),
                    stop=(kt == 7),
                )
                if kt == 7:
                    nc.vector.tensor_copy(qpT[:, mt, :], qp_ps.pop(mt))

            for mt in range(2):
                for kt in range(8):
                    qp_mm(mt, kt)
            kv_block(0)

            # ---- attention ----
            # scores+exp in 2-key-chunk units: [128,1024] tiles amortize the
            # ACT/DVE per-instruction access overhead
            es_q = []  # emitted-but-not-consumed exp tiles: (u, h2, es_bf)

            def do_scores_exp(p, u):
                ksrc = [kpv, kdup]
                for h2 in range(2):
                    lo = h2 * DH
                    for j in range(2):
                        kc = 2 * u + j
                        scp = sc_ps.tile([P, QR], f32, tag="sc", name="scp",
                                         bufs=5)
                        nc.tensor.matmul(
                            scp,
                            ksrc[h2][lo : lo + DH, kc * P : (kc + 1) * P],
                            qpT[lo : lo + DH, p, :],
                            start=True,
                            stop=True,
                        )
                        # engine-split exp: ACT takes even key chunks
                        # exactly, DVE the odd ones via Schraudolph (int16
                        # bits of bf16 exp); both stay under PE's rate
                        if j == 0:
                            es = work.tile([P, QR], bf, tag="es", name="es",
                                           bufs=10)
                            nc.scalar.activation(es, scp, Exp)
                        else:
                            e16 = work.tile([P, QR], i16, tag="es",
                                            name="e16", bufs=10)
                            nc.vector.tensor_scalar(
                                e16, scp, SCHR_A, SCHR_C, Mult, Add
                            )
                            es = e16.bitcast(bf)
                        es_q.append((u, h2, j, es))

            def do_av(av, u, h2, j, es):
                # start=True resets the whole PSUM bank, so only the first
                # group emitted per av tile uses it; the other groups
                # accumulate onto the zeroed bank.
                kc = 2 * u + j
                for qb in range(4):
                    lhsT = es[:, qb * P : (qb + 1) * P]
                    nc.tensor.matmul(
                        av[h2][:, qb, 0:DH],
                        lhsT,
                        vT[:, kc, :],
                        start=(kc == 0 and qb == 0),
                        stop=(kc == KC - 1),
                        skip_group_check=True,
                    )
                    nc.tensor.matmul(
                        av[h2][:, qb, DH : DH + 1],
                        lhsT,
                        ones,
                        start=False,
                        stop=(kc == KC - 1),
                        skip_group_check=True,
                    )

            def drain_av(av, upto):
                # consume queued exp tiles whose unit <= upto
                while es_q and es_q[0][0] <= upto:
                    u, h2, j, es = es_q.pop(0)
                    do_av(av, u, h2, j, es)

            for p in range(8):
                av = [
                    sc_ps.tile([P, 4, DH + 1], f32, tag="av", name="avp",
                               bufs=2)
                    for _ in range(2)
                ]
                for u in range(KC // 2):
                    # ready-first emission: PE dispatch is in-order, so
                    # instructions whose inputs land later go last.
                    # q-proj kt 6+7 both land on u6 so the group's psum
                    # copy clears the single qp bank before the next pair.
                    if p + 2 < 8 and u < 7:
                        qp_mm(p + 2, u)
                        if u == 6:
                            qp_mm(p + 2, 7)
                    if p == 0 and u in (2, 4, 6):
                        kv_block(u // 2)
                    do_scores_exp(p, u)
                    drain_av(av, u - 1)

                # normalize -> A2; batch a = pairs 0-3, b = 4-7. The copy
                # frees the av PSUM bank fast (emitted right after each
                # h2's last av matmul); gpsimd scales from SBUF.
                base = (p // 4) * 4
                col = (p % 4) * P
                for h2 in range(2):
                    # drain this h2's remaining av work first so avsb can
                    # free its bank for the next pair ASAP
                    while es_q and es_q[0][1] == h2:
                        uu, hh, jj, es = es_q.pop(0)
                        do_av(av, uu, hh, jj, es)
                    avsb = work.tile([P, 4, DH + 1], f32, tag="avsb",
                                     name="avsb", bufs=4)
                    if p >= 5:
                        # late pairs: DVE is the busier engine; ACT copies
                        nc.scalar.activation(avsb, av[h2], Copy)
                    else:
                        nc.vector.tensor_copy(avsb, av[h2])
                    rcp = work.tile([P, 4], f32, tag="rcp", name="rcp", bufs=8)
                    nc.vector.reciprocal(
                        rcp, avsb[:, :, DH]
                    )
                    for qb in range(4):
                        dst = A2[base + qb][:, col + h2 * DH : col + (h2 + 1) * DH]
                        nc.gpsimd.tensor_scalar(
                            dst, avsb[:, qb, 0:DH],
                            rcp[:, qb : qb + 1], None, Mult
                        )
                # eager per-(pair, qb) transpose keeps only pair 7's four
                # transposes on the tail critical path
                for qb in range(4):
                    nc.sync.dma_start_transpose(
                        outT[base + qb][:, p % 4, :],
                        A2[base + qb][:, col : col + P],
                    )
                if p == 3:
                    # wc lands mid-kernel, long before the final matmuls,
                    # without its transfer blocking startup-critical DMAs
                    nc.sync.dma_start(
                        wc_sb, wc.rearrange("p (k m) -> p k m", k=8)
                    )

            # ---- final: out[qb] [128q, 1024] = A @ w_concat ----
            for qb in range(4):
                osb = work.tile([P, DM], bf, tag="osb", name="osb", bufs=2)
                for n in range(2):
                    # alternate between the freed qp bank and the av slots
                    if (2 * qb + n) % 2 == 0:
                        fp = sc_ps.tile([P, QR], f32, name="fp", bufs=1,
                                        tag="qp")
                    else:
                        fp = sc_ps.tile([P, QR], f32, name="fp", bufs=2,
                                        tag="av")
                    for g in range(8):
                        nc.tensor.matmul(
                            fp,
                            outT[(g // 4) * 4 + qb][:, g % 4, :],
                            wc_sb[:, g, n * QR : (n + 1) * QR],
                            start=(g == 0),
                            stop=(g == 7),
                        )
                    if n == 0:
                        nc.scalar.activation(
                            osb[:, n * QR : (n + 1) * QR], fp, Copy
                        )
                    else:
                        nc.vector.tensor_copy(osb[:, n * QR : (n + 1) * QR], fp)
                    nc.sync.dma_start(
                        out[qb][:, n * QR : (n + 1) * QR],
                        osb[:, n * QR : (n + 1) * QR],
                    )

    nc.compile()
    return nc


def _get_nc():
    if "nc" not in _CACHE:
        _CACHE["nc"] = _build_bass()
    return _CACHE["nc"]


def make_in_maps(q, kv, w_q, w_kv, w_concat):
    q = np.asarray(q, np.float32)
    kv = np.asarray(kv, np.float32)
    w_qs = (np.asarray(w_q, np.float32) * 0.125).astype(BF)
    w_kvb = np.asarray(w_kv, np.float32).astype(BF)
    w_cb = np.asarray(w_concat, np.float32).astype(BF)

    # wq[mt, p, kt, m] = w_qs[kt*128+p, mt*128+m]
    wq_t = np.ascontiguousarray(
        w_qs.reshape(8, P, 8, P).transpose(2, 1, 0, 3)
    )
    # wkv[p, kt, m] = w_kv[kt*128+p, m]
    wkv_t = np.ascontiguousarray(w_kvb.reshape(8, P, P).transpose(1, 0, 2))
    # wc[p, kt*1024 + n] = w_concat[kt*128+p, n]
    wc_t = np.ascontiguousarray(
        w_cb.reshape(8, P, DM).transpose(1, 0, 2)
    ).reshape(P, 8 * DM)
    # kvt[nt, kt, p, m] = kv[b].T[kt*128+p, nt*512+m]
    kvt_b = []
    for b in range(B):
        kvT = kv[b].T.astype(BF)  # [1024, 2048]
        kvt_b.append(
            np.ascontiguousarray(
                kvT.reshape(8, P, 4, QR).transpose(2, 0, 1, 3)
            )
        )

    in_maps = []
    for c in range(NCORES):
        b, s = c // 4, (c % 4) * QR
        # qT[p, kt*512+m] = q[b, s+m, kt*128+p]
        qs = q[b, s : s + QR, :].T.astype(BF)  # [1024, 512]
        qT_t = np.ascontiguousarray(
            qs.reshape(8, P, QR).transpose(1, 0, 2)
        ).reshape(P, 8 * QR)
        in_maps.append(
            {
                "qT": qT_t,
                "kvt": kvt_b[b],
                "wq": wq_t,
                "wkv": wkv_t,
                "wc": wc_t,
            }
        )
    return in_maps


def assemble(results):
    full = np.empty((B, L, DM), np.float32)
    for c in range(NCORES):
        b, s = c // 4, (c % 4) * QR
        o = np.asarray(results[c]["out"]).astype(np.float32)  # [4, 128, 1024]
        full[b, s : s + QR, :] = o.reshape(QR, DM)
    return full


def kernel(q, kv, w_q, w_kv, w_concat):
    from concourse.bass_utils import run_bass_kernel_spmd

    nc = _get_nc()
    in_maps = make_in_maps(q, kv, w_q, w_kv, w_concat)
    res = run_bass_kernel_spmd(nc, in_maps, core_ids=list(range(NCORES)))
    return assemble(res.results)


# revision 46
# speedup vs baseline: 1.0381x; 1.0032x over previous
"""MQA attention kernel for Trainium2, 8 NeuronCores.

Problem: q,kv [2,2048,1024]; w_q [1024,1024]; w_kv [1024,128]; w_concat
[1024,1024]; 16 heads, d_head 64, shared single K/V head (MQA).

Sharding: queries over L. Core c handles batch c//4, query rows
(c%4)*512..+512 against the full 2048 keys of its batch. Output rows are
disjoint -> no collective.

Data path is bf16 end to end (inputs converted on host, halves DMA; PE
accumulates fp32 in PSUM). Per-core engine budget that this schedule is
built around (cost-model ns):
  PE   ~116us: scores 54.6 + attn@v 27.7 + q-proj 13.7 + w_concat 13.7
               + kv-proj 6.8 (all matmul cost = out_free_size * 0.417ns)
  ACT  ~100us: exp for 11 of 16 heads ([128,512] tiles)
  DVE  ~82us:  exp for 5 heads via one-instruction Schraudolph
               (int16(s*184.66 + C) bitcast to bf16, ~1.8% RMS on those
               heads), PSUM->SBUF copies, softmax normalization
  DMA  ~35us:  9.25MB in + 1MB out + XBAR transposes

Layout choices:
  - scores.T [keys, queries] per (head, key-chunk): lhsT = k chunk,
    rhs = qp head slice, out [128k, 512q] -- full 128-partition output.
  - attn@v reoriented to out [128q, 64d]: lhsT = exp-scores [128k, 128q]
    slice, rhs = vT[:, kc, :] -- N=64 instead of N=512 halves av PE cost.
    Softmax denominators via an extra N=1 matmul against a ones column.
  - vT built by XBAR dma transposes of the kv projection (no PE/DVE).
  - attention out assembled per qblock in SBUF bf16 [128q, 512(4 pairs)],
    XBAR-transposed to [128dm, 4, 128q] for the final w_concat matmul.
  - k duplicated to partitions 64:128 via gpsimd SWDGE so each head pair
    reads k/qp at matching partition bases.
PSUM: sc x3 + qp x2 + kvp x1 + av x2 = 8 banks.
"""

import numpy as np
import ml_dtypes

B, L, DM = 2, 2048, 1024
H, DH = 16, 64
NCORES = 8
QR = 512          # query rows per core
P = 128
KC = 16           # key chunks of 128
BF = ml_dtypes.bfloat16

# per-h2 column split of each [128,1024] exp tile: ACT takes [0:x] exactly,
# DVE takes [x:1024] via Schraudolph (7/16 of elements)
ACT_COLS = (640, 512)
SCHR_A = 184.66496523378732   # 128 / ln 2
SCHR_C = 16248.5              # tuned for scores ~ N(0, 0.41^2)
PIPE = 2                      # av lags scores by this many key chunks

_CACHE = {}


def _build_bass():
    import concourse.mybir as mybir
    import concourse.tile as tile
    from concourse import bacc

    f32 = mybir.dt.float32
    bf = mybir.dt.bfloat16
    i16 = mybir.dt.int16
    Exp = mybir.ActivationFunctionType.Exp
    Copy = mybir.ActivationFunctionType.Copy
    Mult = mybir.AluOpType.mult
    Add = mybir.AluOpType.add
    Div = mybir.AluOpType.divide

    nc = bacc.Bacc(
        "TRN2", target_bir_lowering=False, debug=False, enable_asserts=True
    )

    qT = nc.dram_tensor("qT", [P, 8 * QR], bf, kind="ExternalInput").ap()
    kvt = nc.dram_tensor("kvt", [4, 8, P, QR], bf, kind="ExternalInput").ap()
    wq = nc.dram_tensor("wq", [8, P, 8, P], bf, kind="ExternalInput").ap()
    wkv = nc.dram_tensor("wkv", [P, 8, P], bf, kind="ExternalInput").ap()
    wc = nc.dram_tensor("wc", [P, 8 * DM], bf, kind="ExternalInput").ap()
    out = nc.dram_tensor("out", [4, P, DM], bf, kind="ExternalOutput").ap()

    with tile.TileContext(nc) as tc:
        with (
            tc.tile_pool(name="persist", bufs=1) as persist,
            tc.tile_pool(name="kvs", bufs=9) as kvs,
            tc.tile_pool(name="work", bufs=1) as work,
            tc.tile_pool(name="sc_ps", bufs=1, space="PSUM") as sc_ps,
        ):
            kpv = persist.tile([P, L], bf, name="kpv")    # k rows 0:64, v 64:128
            kdup = persist.tile([P, L], bf, name="kdup")  # k dup at rows 64:128
            vT = persist.tile([P, KC, DH], bf, name="vT")
            ones = persist.tile([P, 1], bf, name="ones")
            qpT = persist.tile([P, 8, QR], bf, name="qpT")
            wkv_sb = persist.tile([P, 8, P], bf, name="wkv_sb")
            wq_sb = [
                persist.tile([P, 8, P], bf, name=f"wq_sb{mt}") for mt in range(8)
            ]
            qT_sb = persist.tile([P, 8, QR], bf, name="qT_sb")
            wc_sb = persist.tile([P, 8, DM], bf, name="wc_sb")
            # attn out per qblock: [128q, 4 pairs * 128]; batch a = pairs 0-3
            A2 = [persist.tile([P, 4 * P], bf, name=f"A2_{i}") for i in range(8)]
            outT = [
                persist.tile([P, 4, P], bf, name=f"outT_{i}") for i in range(8)
            ]

            nc.gpsimd.memset(ones, 1.0)

            # ---- input DMAs in priority order ----
            kv_chunks = {}

            def dma_kv_block(nt):
                ch = kvs.tile([P, 8, QR], bf, tag="kv", name="kv_ch", bufs=4)
                nc.sync.dma_start(ch, kvt[nt].rearrange("k p m -> p k m"))
                kv_chunks[nt] = ch

            # DMA order tuned so each PE instruction's inputs land just
            # before the in-order PE dispatch reaches it: q-projection
            # inputs first (PE's first work), all kv blocks before the
            # late-pair wq groups, wc deferred to mid-kernel (its 5.8us
            # transfer must not delay vT transposes on the DMA engines).
            qT3 = qT.rearrange("p (k m) -> p k m", k=8)
            nc.sync.dma_start(wq_sb[0], wq[0])
            # qT in halves: the first q-proj matmuls start after 0.5MB
            nc.sync.dma_start(qT_sb[:, 0:4, :], qT3[:, 0:4, :])
            nc.sync.dma_start(qT_sb[:, 4:8, :], qT3[:, 4:8, :])
            nc.sync.dma_start(wq_sb[1], wq[1])
            nc.sync.dma_start(wkv_sb, wkv)
            dma_kv_block(0)
            dma_kv_block(1)
            nc.sync.dma_start(wq_sb[2], wq[2])
            dma_kv_block(2)
            dma_kv_block(3)
            nc.sync.dma_start(wq_sb[3], wq[3])
            nc.sync.dma_start(wq_sb[4], wq[4])
            nc.sync.dma_start(wq_sb[5], wq[5])
            nc.sync.dma_start(wq_sb[6], wq[6])
            nc.sync.dma_start(wq_sb[7], wq[7])

            # ---- kv projection per 512-col block ----
            def kv_block(nt):
                sl = slice(nt * QR, (nt + 1) * QR)
                ps = sc_ps.tile([P, QR], f32, tag="sc", name="ps_kv", bufs=5)
                ch = kv_chunks.pop(nt)
                for kt in range(8):
                    nc.tensor.matmul(
                        ps,
                        wkv_sb[:, kt, :],
                        ch[:, kt, :],
                        start=(kt == 0),
                        stop=(kt == 7),
                    )
                nc.vector.tensor_copy(kpv[:, sl], ps)
                # k dup to rows 64:128 (partition shift via SWDGE)
                nc.gpsimd.dma_start(kdup[DH : 2 * DH, sl], kpv[0:DH, sl])
                # vT[:, 4nt:4nt+4, :] = transpose of v rows (XBAR writes the
                # 256-elem slice contiguously)
                nc.sync.dma_start_transpose(
                    vT[:, 4 * nt : 4 * nt + 4, :], kpv[DH : 2 * DH, sl]
                )

            # ---- q projection: groups 0,1 upfront; 2..7 spread below ----
            qp_ps = {}

            def qp_mm(mt, kt):
                if kt == 0:
                    qp_ps[mt] = sc_ps.tile(
                        [P, QR], f32, tag="qp", name="ps_q", bufs=1
                    )
                nc.tensor.matmul(
                    qp_ps[mt],
                    wq_sb[mt][:, kt, :],
                    qT_sb[:, kt, :],
                    start=(kt == 0),
                    stop=(kt == 7),
                )
                if kt == 7:
                    nc.vector.tensor_copy(qpT[:, mt, :], qp_ps.pop(mt))

            for mt in range(2):
                for kt in range(8):
                    qp_mm(mt, kt)
            kv_block(0)

            # ---- attention ----
            # scores+exp in 2-key-chunk units: [128,1024] tiles amortize the
            # ACT/DVE per-instruction access overhead
            es_q = []  # emitted-but-not-consumed exp tiles: (u, h2, es_bf)

            def do_scores_exp(p, u):
                ksrc = [kpv, kdup]
                for h2 in range(2):
                    lo = h2 * DH
                    for j in range(2):
                        kc = 2 * u + j
                        scp = sc_ps.tile([P, QR], f32, tag="sc", name="scp",
                                         bufs=5)
                        nc.tensor.matmul(
                            scp,
                            ksrc[h2][lo : lo + DH, kc * P : (kc + 1) * P],
                            qpT[lo : lo + DH, p, :],
                            start=True,
                            stop=True,
                        )
                        # engine-split exp: ACT takes even key chunks
                        # exactly, DVE the odd ones via Schraudolph (int16
                        # bits of bf16 exp); both stay under PE's rate
                        if j == 0:
                            es = work.tile([P, QR], bf, tag="es", name="es",
                                           bufs=10)
                            nc.scalar.activation(es, scp, Exp)
                        else:
                            e16 = work.tile([P, QR], i16, tag="es",
                                            name="e16", bufs=10)
                            nc.vector.tensor_scalar(
                                e16, scp, SCHR_A, SCHR_C, Mult, Add
                            )
                            es = e16.bitcast(bf)
                        es_q.append((u, h2, j, es))

            def do_av(av, u, h2, j, es):
                # start=True resets the whole PSUM bank, so only the first
                # group emitted per av tile uses it; the other groups
                # accumulate onto the zeroed bank.
                kc = 2 * u + j
                for qb in range(4):
                    lhsT = es[:, qb * P : (qb + 1) * P]
                    nc.tensor.matmul(
                        av[h2][:, qb, 0:DH],
                        lhsT,
                        vT[:, kc, :],
                        start=(kc == 0 and qb == 0),
                        stop=(kc == KC - 1),
                        skip_group_check=True,
                    )
                    nc.tensor.matmul(
                        av[h2][:, qb, DH : DH + 1],
                        lhsT,
                        ones,
                        start=False,
                        stop=(kc == KC - 1),
                        skip_group_check=True,
                    )

            def drain_av(av, upto):
                # consume queued exp tiles whose unit <= upto
                while es_q and es_q[0][0] <= upto:
                    u, h2, j, es = es_q.pop(0)
                    do_av(av, u, h2, j, es)

            for p in range(8):
                av = [
                    sc_ps.tile([P, 4, DH + 1], f32, tag="av", name="avp",
                               bufs=2)
                    for _ in range(2)
                ]
                for u in range(KC // 2):
                    # ready-first emission: PE dispatch is in-order, so
                    # instructions whose inputs land later go last.
                    # q-proj kt 6+7 both land on u6 so the group's psum
                    # copy clears the single qp bank before the next pair.
                    if p + 2 < 8 and u < 7:
                        qp_mm(p + 2, u)
                        if u == 6:
                            qp_mm(p + 2, 7)
                    if p == 0 and u in (2, 4, 6):
                        kv_block(u // 2)
                    do_scores_exp(p, u)
                    drain_av(av, u - 1)

                # normalize -> A2; batch a = pairs 0-3, b = 4-7. The copy
                # frees the av PSUM bank fast (emitted right after each
                # h2's last av matmul); gpsimd scales from SBUF.
                base = (p // 4) * 4
                col = (p % 4) * P
                for h2 in range(2):
                    # drain this h2's remaining av work first so avsb can
                    # free its bank for the next pair ASAP
                    while es_q and es_q[0][1] == h2:
                        uu, hh, jj, es = es_q.pop(0)
                        do_av(av, uu, hh, jj, es)
                    avsb = work.tile([P, 4, DH + 1], f32, tag="avsb",
                                     name="avsb", bufs=4)
                    if p >= 5:
                        # late pairs: DVE is the busier engine; ACT copies
                        nc.scalar.activation(avsb, av[h2], Copy)
                    else:
                        nc.vector.tensor_copy(avsb, av[h2])
                    rcp = work.tile([P, 4], f32, tag="rcp", name="rcp", bufs=8)
                    nc.vector.reciprocal(
                        rcp, avsb[:, :, DH]
                    )
                    for qb in range(4):
                        dst = A2[base + qb][:, col + h2 * DH : col + (h2 + 1) * DH]
                        nc.gpsimd.tensor_scalar(
                            dst, avsb[:, qb, 0:DH],
                            rcp[:, qb : qb + 1], None, Mult
                        )
                # eager per-(pair, qb) transpose keeps only pair 7's four
                # transposes on the tail critical path
                for qb in range(4):
                    nc.sync.dma_start_transpose(
                        outT[base + qb][:, p % 4, :],
                        A2[base + qb][:, col : col + P],
                    )
                if p == 3:
                    # wc lands mid-kernel, long before the final matmuls,
                    # without its transfer blocking startup-critical DMAs
                    nc.sync.dma_start(
                        wc_sb, wc.rearrange("p (k m) -> p k m", k=8)
                    )

            # ---- final: out[qb] [128q, 1024] = A @ w_concat ----
            for qb in range(4):
                osb = work.tile([P, DM], bf, tag="osb", name="osb", bufs=2)
                for n in range(2):
                    # alternate between the freed qp bank and the av slots
                    if (2 * qb + n) % 2 == 0:
                        fp = sc_ps.tile([P, QR], f32, name="fp", bufs=1,
                                        tag="qp")
                    else:
                        fp = sc_ps.tile([P, QR], f32, name="fp", bufs=2,
                                        tag="av")
                    for g in range(8):
                        nc.tensor.matmul(
                            fp,
                            outT[(g // 4) * 4 + qb][:, g % 4, :],
                            wc_sb[:, g, n * QR : (n + 1) * QR],
                            start=(g == 0),
                            stop=(g == 7),
                        )
                    if n == 0:
                        nc.scalar.activation(
                            osb[:, n * QR : (n + 1) * QR], fp, Copy
                        )
                    else:
                        nc.vector.tensor_copy(osb[:, n * QR : (n + 1) * QR], fp)
                    nc.sync.dma_start(
                        out[qb][:, n * QR : (n + 1) * QR],
                        osb[:, n * QR : (n + 1) * QR],
                    )

    nc.compile()
    return nc


def _get_nc():
    if "nc" not in _CACHE:
        _CACHE["nc"] = _build_bass()
    return _CACHE["nc"]


def make_in_maps(q, kv, w_q, w_kv, w_concat):
    q = np.asarray(q, np.float32)
    kv = np.asarray(kv, np.float32)
    w_qs = (np.asarray(w_q, np.float32) * 0.125).astype(BF)
    w_kvb = np.asarray(w_kv, np.float32).astype(BF)
    w_cb = np.asarray(w_concat, np.float32).astype(BF)

    # wq[mt, p, kt, m] = w_qs[kt*128+p, mt*128+m]
    wq_t = np.ascontiguousarray(
        w_qs.reshape(8, P, 8, P).transpose(2, 1, 0, 3)
    )
    # wkv[p, kt, m] = w_kv[kt*128+p, m]
    wkv_t = np.ascontiguousarray(w_kvb.reshape(8, P, P).transpose(1, 0, 2))
    # wc[p, kt*1024 + n] = w_concat[kt*128+p, n]
    wc_t = np.ascontiguousarray(
        w_cb.reshape(8, P, DM).transpose(1, 0, 2)
    ).reshape(P, 8 * DM)
    # kvt[nt, kt, p, m] = kv[b].T[kt*128+p, nt*512+m]
    kvt_b = []
    for b in range(B):
        kvT = kv[b].T.astype(BF)  # [1024, 2048]
        kvt_b.append(
            np.ascontiguousarray(
                kvT.reshape(8, P, 4, QR).transpose(2, 0, 1, 3)
            )
        )

    in_maps = []
    for c in range(NCORES):
        b, s = c // 4, (c % 4) * QR
        # qT[p, kt*512+m] = q[b, s+m, kt*128+p]
        qs = q[b, s : s + QR, :].T.astype(BF)  # [1024, 512]
        qT_t = np.ascontiguousarray(
            qs.reshape(8, P, QR).transpose(1, 0, 2)
        ).reshape(P, 8 * QR)
        in_maps.append(
            {
                "qT": qT_t,
                "kvt": kvt_b[b],
                "wq": wq_t,
                "wkv": wkv_t,
                "wc": wc_t,
            }
        )
    return in_maps


def assemble(results):
    full = np.empty((B, L, DM), np.float32)
    for c in range(NCORES):
        b, s = c // 4, (c % 4) * QR
        o = np.asarray(results[c]["out"]).astype(np.float32)  # [4, 128, 1024]
        full[b, s : s + QR, :] = o.reshape(QR, DM)
    return full


def kernel(q, kv, w_q, w_kv, w_concat):
    from concourse.bass_utils import run_bass_kernel_spmd

    nc = _get_nc()
    in_maps = make_in_maps(q, kv, w_q, w_kv, w_concat)
    res = run_bass_kernel_spmd(nc, in_maps, core_ids=list(range(NCORES)))
    return assemble(res.results)


# revision 47
# speedup vs baseline: 1.0449x; 1.0066x over previous
"""MQA attention kernel for Trainium2, 8 NeuronCores.

Problem: q,kv [2,2048,1024]; w_q [1024,1024]; w_kv [1024,128]; w_concat
[1024,1024]; 16 heads, d_head 64, shared single K/V head (MQA).

Sharding: queries over L. Core c handles batch c//4, query rows
(c%4)*512..+512 against the full 2048 keys of its batch. Output rows are
disjoint -> no collective.

Data path is bf16 end to end (inputs converted on host, halves DMA; PE
accumulates fp32 in PSUM). Per-core engine budget that this schedule is
built around (cost-model ns):
  PE   ~116us: scores 54.6 + attn@v 27.7 + q-proj 13.7 + w_concat 13.7
               + kv-proj 6.8 (all matmul cost = out_free_size * 0.417ns)
  ACT  ~100us: exp for 11 of 16 heads ([128,512] tiles)
  DVE  ~82us:  exp for 5 heads via one-instruction Schraudolph
               (int16(s*184.66 + C) bitcast to bf16, ~1.8% RMS on those
               heads), PSUM->SBUF copies, softmax normalization
  DMA  ~35us:  9.25MB in + 1MB out + XBAR transposes

Layout choices:
  - scores.T [keys, queries] per (head, key-chunk): lhsT = k chunk,
    rhs = qp head slice, out [128k, 512q] -- full 128-partition output.
  - attn@v reoriented to out [128q, 64d]: lhsT = exp-scores [128k, 128q]
    slice, rhs = vT[:, kc, :] -- N=64 instead of N=512 halves av PE cost.
    Softmax denominators via an extra N=1 matmul against a ones column.
  - vT built by XBAR dma transposes of the kv projection (no PE/DVE).
  - attention out assembled per qblock in SBUF bf16 [128q, 512(4 pairs)],
    XBAR-transposed to [128dm, 4, 128q] for the final w_concat matmul.
  - k duplicated to partitions 64:128 via gpsimd SWDGE so each head pair
    reads k/qp at matching partition bases.
PSUM: sc x3 + qp x2 + kvp x1 + av x2 = 8 banks.
"""

import numpy as np
import ml_dtypes

B, L, DM = 2, 2048, 1024
H, DH = 16, 64
NCORES = 8
QR = 512          # query rows per core
P = 128
KC = 16           # key chunks of 128
BF = ml_dtypes.bfloat16

# per-h2 column split of each [128,1024] exp tile: ACT takes [0:x] exactly,
# DVE takes [x:1024] via Schraudolph (7/16 of elements)
ACT_COLS = (640, 512)
SCHR_A = 184.66496523378732   # 128 / ln 2
SCHR_C = 16248.5              # tuned for scores ~ N(0, 0.41^2)
PIPE = 2                      # av lags scores by this many key chunks

_CACHE = {}


def _build_bass():
    import concourse.mybir as mybir
    import concourse.tile as tile
    from concourse import bacc

    f32 = mybir.dt.float32
    bf = mybir.dt.bfloat16
    i16 = mybir.dt.int16
    Exp = mybir.ActivationFunctionType.Exp
    Copy = mybir.ActivationFunctionType.Copy
    Mult = mybir.AluOpType.mult
    Add = mybir.AluOpType.add
    Div = mybir.AluOpType.divide

    nc = bacc.Bacc(
        "TRN2", target_bir_lowering=False, debug=False, enable_asserts=True
    )

    qT = nc.dram_tensor("qT", [P, 8 * QR], bf, kind="ExternalInput").ap()
    kvt = nc.dram_tensor("kvt", [4, 8, P, QR], bf, kind="ExternalInput").ap()
    wq = nc.dram_tensor("wq", [8, P, 8, P], bf, kind="ExternalInput").ap()
    wkv = nc.dram_tensor("wkv", [P, 8, P], bf, kind="ExternalInput").ap()
    wc = nc.dram_tensor("wc", [P, 8 * DM], bf, kind="ExternalInput").ap()
    out = nc.dram_tensor("out", [4, P, DM], bf, kind="ExternalOutput").ap()

    with tile.TileContext(nc) as tc:
        with (
            tc.tile_pool(name="persist", bufs=1) as persist,
            tc.tile_pool(name="kvs", bufs=9) as kvs,
            tc.tile_pool(name="work", bufs=1) as work,
            tc.tile_pool(name="sc_ps", bufs=1, space="PSUM") as sc_ps,
        ):
            kpv = persist.tile([P, L], bf, name="kpv")    # k rows 0:64, v 64:128
            kdup = persist.tile([P, L], bf, name="kdup")  # k dup at rows 64:128
            vT = persist.tile([P, KC, DH], bf, name="vT")
            ones = persist.tile([P, 1], bf, name="ones")
            qpT = persist.tile([P, 8, QR], bf, name="qpT")
            wkv_sb = persist.tile([P, 8, P], bf, name="wkv_sb")
            wq_sb = [
                persist.tile([P, 8, P], bf, name=f"wq_sb{mt}") for mt in range(8)
            ]
            qT_sb = persist.tile([P, 8, QR], bf, name="qT_sb")
            wc_sb = persist.tile([P, 8, DM], bf, name="wc_sb")
            # attn out per qblock: [128q, 4 pairs * 128]; batch a = pairs 0-3
            A2 = [persist.tile([P, 4 * P], bf, name=f"A2_{i}") for i in range(8)]
            outT = [
                persist.tile([P, 4, P], bf, name=f"outT_{i}") for i in range(8)
            ]

            nc.gpsimd.memset(ones, 1.0)

            # ---- input DMAs in priority order ----
            kv_chunks = {}

            def dma_kv_block(nt):
                ch = kvs.tile([P, 8, QR], bf, tag="kv", name="kv_ch", bufs=4)
                nc.sync.dma_start(ch, kvt[nt].rearrange("k p m -> p k m"))
                kv_chunks[nt] = ch

            # DMA order tuned so each PE instruction's inputs land just
            # before the in-order PE dispatch reaches it: q-projection
            # inputs first (PE's first work), all kv blocks before the
            # late-pair wq groups, wc deferred to mid-kernel (its 5.8us
            # transfer must not delay vT transposes on the DMA engines).
            qT3 = qT.rearrange("p (k m) -> p k m", k=8)
            nc.sync.dma_start(wq_sb[0], wq[0])
            # qT in halves: the first q-proj matmuls start after 0.5MB
            nc.sync.dma_start(qT_sb[:, 0:4, :], qT3[:, 0:4, :])
            nc.sync.dma_start(qT_sb[:, 4:8, :], qT3[:, 4:8, :])
            nc.sync.dma_start(wq_sb[1], wq[1])
            nc.sync.dma_start(wq_sb[2], wq[2])
            nc.sync.dma_start(wkv_sb, wkv)
            dma_kv_block(0)
            dma_kv_block(1)
            dma_kv_block(2)
            dma_kv_block(3)
            nc.sync.dma_start(wq_sb[3], wq[3])
            nc.sync.dma_start(wq_sb[4], wq[4])
            nc.sync.dma_start(wq_sb[5], wq[5])
            nc.sync.dma_start(wq_sb[6], wq[6])
            nc.sync.dma_start(wq_sb[7], wq[7])

            # ---- kv projection per 512-col block ----
            def kv_block(nt):
                sl = slice(nt * QR, (nt + 1) * QR)
                ps = sc_ps.tile([P, QR], f32, tag="sc", name="ps_kv", bufs=5)
                ch = kv_chunks.pop(nt)
                for kt in range(8):
                    nc.tensor.matmul(
                        ps,
                        wkv_sb[:, kt, :],
                        ch[:, kt, :],
                        start=(kt == 0),
                        stop=(kt == 7),
                    )
                nc.vector.tensor_copy(kpv[:, sl], ps)
                # k dup to rows 64:128 (partition shift via SWDGE)
                nc.gpsimd.dma_start(kdup[DH : 2 * DH, sl], kpv[0:DH, sl])
                # vT[:, 4nt:4nt+4, :] = transpose of v rows (XBAR writes the
                # 256-elem slice contiguously)
                nc.sync.dma_start_transpose(
                    vT[:, 4 * nt : 4 * nt + 4, :], kpv[DH : 2 * DH, sl]
                )

            # ---- q projection: groups 0,1 upfront; 2..7 spread below ----
            qp_ps = {}

            def qp_mm(mt, kt):
                if kt == 0:
                    qp_ps[mt] = sc_ps.tile(
                        [P, QR], f32, tag="qp", name="ps_q", bufs=1
                    )
                nc.tensor.matmul(
                    qp_ps[mt],
                    wq_sb[mt][:, kt, :],
                    qT_sb[:, kt, :],
                    start=(kt == 0),
                    stop=(kt == 7),
                )
                if kt == 7:
                    nc.vector.tensor_copy(qpT[:, mt, :], qp_ps.pop(mt))

            for mt in range(2):
                for kt in range(8):
                    qp_mm(mt, kt)
            kv_block(0)

            # ---- attention ----
            # scores+exp in 2-key-chunk units: [128,1024] tiles amortize the
            # ACT/DVE per-instruction access overhead
            es_q = []  # emitted-but-not-consumed exp tiles: (u, h2, es_bf)

            def do_scores_exp(p, u):
                ksrc = [kpv, kdup]
                for h2 in range(2):
                    lo = h2 * DH
                    for j in range(2):
                        kc = 2 * u + j
                        scp = sc_ps.tile([P, QR], f32, tag="sc", name="scp",
                                         bufs=5)
                        nc.tensor.matmul(
                            scp,
                            ksrc[h2][lo : lo + DH, kc * P : (kc + 1) * P],
                            qpT[lo : lo + DH, p, :],
                            start=True,
                            stop=True,
                        )
                        # engine-split exp: ACT takes even key chunks
                        # exactly, DVE the odd ones via Schraudolph (int16
                        # bits of bf16 exp); both stay under PE's rate
                        if j == 0:
                            es = work.tile([P, QR], bf, tag="es", name="es",
                                           bufs=10)
                            nc.scalar.activation(es, scp, Exp)
                        else:
                            e16 = work.tile([P, QR], i16, tag="es",
                                            name="e16", bufs=10)
                            nc.vector.tensor_scalar(
                                e16, scp, SCHR_A, SCHR_C, Mult, Add
                            )
                            es = e16.bitcast(bf)
                        es_q.append((u, h2, j, es))

            def do_av(av, u, h2, j, es):
                # start=True resets the whole PSUM bank, so only the first
                # group emitted per av tile uses it; the other groups
                # accumulate onto the zeroed bank.
                kc = 2 * u + j
                for qb in range(4):
                    lhsT = es[:, qb * P : (qb + 1) * P]
                    nc.tensor.matmul(
                        av[h2][:, qb, 0:DH],
                        lhsT,
                        vT[:, kc, :],
                        start=(kc == 0 and qb == 0),
                        stop=(kc == KC - 1),
                        skip_group_check=True,
                    )
                    nc.tensor.matmul(
                        av[h2][:, qb, DH : DH + 1],
                        lhsT,
                        ones,
                        start=False,
                        stop=(kc == KC - 1),
                        skip_group_check=True,
                    )

            def drain_av(av, upto):
                # consume queued exp tiles whose unit <= upto
                while es_q and es_q[0][0] <= upto:
                    u, h2, j, es = es_q.pop(0)
                    do_av(av, u, h2, j, es)

            for p in range(8):
                av = [
                    sc_ps.tile([P, 4, DH + 1], f32, tag="av", name="avp",
                               bufs=2)
                    for _ in range(2)
                ]
                for u in range(KC // 2):
                    # ready-first emission: PE dispatch is in-order, so
                    # instructions whose inputs land later go last.
                    # q-proj kt 6+7 both land on u6 so the group's psum
                    # copy clears the single qp bank before the next pair.
                    if p + 2 < 8 and u < 7:
                        qp_mm(p + 2, u)
                        if u == 6:
                            qp_mm(p + 2, 7)
                    if p == 0 and u in (2, 4, 6):
                        kv_block(u // 2)
                    do_scores_exp(p, u)
                    drain_av(av, u - 1)

                # normalize -> A2; batch a = pairs 0-3, b = 4-7. The copy
                # frees the av PSUM bank fast (emitted right after each
                # h2's last av matmul); gpsimd scales from SBUF.
                base = (p // 4) * 4
                col = (p % 4) * P
                for h2 in range(2):
                    # drain this h2's remaining av work first so avsb can
                    # free its bank for the next pair ASAP
                    while es_q and es_q[0][1] == h2:
                        uu, hh, jj, es = es_q.pop(0)
                        do_av(av, uu, hh, jj, es)
                    avsb = work.tile([P, 4, DH + 1], f32, tag="avsb",
                                     name="avsb", bufs=4)
                    if p >= 5:
                        # late pairs: DVE is the busier engine; ACT copies
                        nc.scalar.activation(avsb, av[h2], Copy)
                    else:
                        nc.vector.tensor_copy(avsb, av[h2])
                    rcp = work.tile([P, 4], f32, tag="rcp", name="rcp", bufs=8)
                    nc.vector.reciprocal(
                        rcp, avsb[:, :, DH]
                    )
                    for qb in range(4):
                        dst = A2[base + qb][:, col + h2 * DH : col + (h2 + 1) * DH]
                        nc.gpsimd.tensor_scalar(
                            dst, avsb[:, qb, 0:DH],
                            rcp[:, qb : qb + 1], None, Mult
                        )
                # eager per-(pair, qb) transpose keeps only pair 7's four
                # transposes on the tail critical path
                for qb in range(4):
                    nc.sync.dma_start_transpose(
                        outT[base + qb][:, p % 4, :],
                        A2[base + qb][:, col : col + P],
                    )
                if p == 3:
                    # wc lands mid-kernel, long before the final matmuls,
                    # without its transfer blocking startup-critical DMAs
                    nc.sync.dma_start(
                        wc_sb, wc.rearrange("p (k m) -> p k m", k=8)
                    )

            # ---- final: out[qb] [128q, 1024] = A @ w_concat ----
            for qb in range(4):
                osb = work.tile([P, DM], bf, tag="osb", name="osb", bufs=2)
                for n in range(2):
                    # alternate between the freed qp bank and the av slots
                    if (2 * qb + n) % 2 == 0:
                        fp = sc_ps.tile([P, QR], f32, name="fp", bufs=1,
                                        tag="qp")
                    else:
                        fp = sc_ps.tile([P, QR], f32, name="fp", bufs=2,
                                        tag="av")
                    for g in range(8):
                        nc.tensor.matmul(
                            fp,
                            outT[(g // 4) * 4 + qb][:, g % 4, :],
                            wc_sb[:, g, n * QR : (n + 1) * QR],
                            start=(g == 0),
                            stop=(g == 7),
                        )
                    if n == 0:
                        nc.scalar.activation(
                            osb[:, n * QR : (n + 1) * QR], fp, Copy
                        )
                    else:
                        nc.vector.tensor_copy(osb[:, n * QR : (n + 1) * QR], fp)
                    nc.sync.dma_start(
                        out[qb][:, n * QR : (n + 1) * QR],
                        osb[:, n * QR : (n + 1) * QR],
                    )

    nc.compile()
    return nc


def _get_nc():
    if "nc" not in _CACHE:
        _CACHE["nc"] = _build_bass()
    return _CACHE["nc"]


def make_in_maps(q, kv, w_q, w_kv, w_concat):
    q = np.asarray(q, np.float32)
    kv = np.asarray(kv, np.float32)
    w_qs = (np.asarray(w_q, np.float32) * 0.125).astype(BF)
    w_kvb = np.asarray(w_kv, np.float32).astype(BF)
    w_cb = np.asarray(w_concat, np.float32).astype(BF)

    # wq[mt, p, kt, m] = w_qs[kt*128+p, mt*128+m]
    wq_t = np.ascontiguousarray(
        w_qs.reshape(8, P, 8, P).transpose(2, 1, 0, 3)
    )
    # wkv[p, kt, m] = w_kv[kt*128+p, m]
    wkv_t = np.ascontiguousarray(w_kvb.reshape(8, P, P).transpose(1, 0, 2))
    # wc[p, kt*1024 + n] = w_concat[kt*128+p, n]
    wc_t = np.ascontiguousarray(
        w_cb.reshape(8, P, DM).transpose(1, 0, 2)
    ).reshape(P, 8 * DM)
    # kvt[nt, kt, p, m] = kv[b].T[kt*128+p, nt*512+m]
    kvt_b = []
    for b in range(B):
        kvT = kv[b].T.astype(BF)  # [1024, 2048]
        kvt_b.append(
            np.ascontiguousarray(
                kvT.reshape(8, P, 4, QR).transpose(2, 0, 1, 3)
            )
        )

    in_maps = []
    for c in range(NCORES):
        b, s = c // 4, (c % 4) * QR
        # qT[p, kt*512+m] = q[b, s+m, kt*128+p]
        qs = q[b, s : s + QR, :].T.astype(BF)  # [1024, 512]
        qT_t = np.ascontiguousarray(
            qs.reshape(8, P, QR).transpose(1, 0, 2)
        ).reshape(P, 8 * QR)
        in_maps.append(
            {
                "qT": qT_t,
                "kvt": kvt_b[b],
                "wq": wq_t,
                "wkv": wkv_t,
                "wc": wc_t,
            }
        )
    return in_maps


def assemble(results):
    full = np.empty((B, L, DM), np.float32)
    for c in range(NCORES):
        b, s = c // 4, (c % 4) * QR
        o = np.asarray(results[c]["out"]).astype(np.float32)  # [4, 128, 1024]
        full[b, s : s + QR, :] = o.reshape(QR, DM)
    return full


def kernel(q, kv, w_q, w_kv, w_concat):
    from concourse.bass_utils import run_bass_kernel_spmd

    nc = _get_nc()
    in_maps = make_in_maps(q, kv, w_q, w_kv, w_concat)
    res = run_bass_kernel_spmd(nc, in_maps, core_ids=list(range(NCORES)))
    return assemble(res.results)


# revision 50
# speedup vs baseline: 1.0770x; 1.0307x over previous
"""MQA attention kernel for Trainium2, 8 NeuronCores.

Problem: q,kv [2,2048,1024]; w_q [1024,1024]; w_kv [1024,128]; w_concat
[1024,1024]; 16 heads, d_head 64, shared single K/V head (MQA).

Sharding: queries over L. Core c handles batch c//4, query rows
(c%4)*512..+512 against the full 2048 keys of its batch. Output rows are
disjoint -> no collective.

Data path is bf16 end to end (inputs converted on host, halves DMA; PE
accumulates fp32 in PSUM). Per-core engine budget that this schedule is
built around (cost-model ns):
  PE   ~116us: scores 54.6 + attn@v 27.7 + q-proj 13.7 + w_concat 13.7
               + kv-proj 6.8 (all matmul cost = out_free_size * 0.417ns)
  ACT  ~100us: exp for 11 of 16 heads ([128,512] tiles)
  DVE  ~82us:  exp for 5 heads via one-instruction Schraudolph
               (int16(s*184.66 + C) bitcast to bf16, ~1.8% RMS on those
               heads), PSUM->SBUF copies, softmax normalization
  DMA  ~35us:  9.25MB in + 1MB out + XBAR transposes

Layout choices:
  - scores.T [keys, queries] per (head, key-chunk): lhsT = k chunk,
    rhs = qp head slice, out [128k, 512q] -- full 128-partition output.
  - attn@v reoriented to out [128q, 64d]: lhsT = exp-scores [128k, 128q]
    slice, rhs = vT[:, kc, :] -- N=64 instead of N=512 halves av PE cost.
    Softmax denominators via an extra N=1 matmul against a ones column.
  - vT built by XBAR dma transposes of the kv projection (no PE/DVE).
  - attention out assembled per qblock in SBUF bf16 [128q, 512(4 pairs)],
    XBAR-transposed to [128dm, 4, 128q] for the final w_concat matmul.
  - k duplicated to partitions 64:128 via gpsimd SWDGE so each head pair
    reads k/qp at matching partition bases.
PSUM: sc x3 + qp x2 + kvp x1 + av x2 = 8 banks.
"""

import numpy as np
import ml_dtypes

B, L, DM = 2, 2048, 1024
H, DH = 16, 64
NCORES = 8
QR = 512          # query rows per core
P = 128
KC = 16           # key chunks of 128
BF = ml_dtypes.bfloat16

# per-h2 column split of each [128,1024] exp tile: ACT takes [0:x] exactly,
# DVE takes [x:1024] via Schraudolph (7/16 of elements)
ACT_COLS = (640, 512)
SCHR_A = 184.66496523378732   # 128 / ln 2
SCHR_C = 16248.5              # tuned for scores ~ N(0, 0.41^2)
PIPE = 2                      # av lags scores by this many key chunks

_CACHE = {}


def _build_bass():
    import concourse.mybir as mybir
    import concourse.tile as tile
    from concourse import bacc

    f32 = mybir.dt.float32
    bf = mybir.dt.bfloat16
    i16 = mybir.dt.int16
    Exp = mybir.ActivationFunctionType.Exp
    Copy = mybir.ActivationFunctionType.Copy
    Mult = mybir.AluOpType.mult
    Add = mybir.AluOpType.add
    Div = mybir.AluOpType.divide

    nc = bacc.Bacc(
        "TRN2", target_bir_lowering=False, debug=False, enable_asserts=True
    )

    qT = nc.dram_tensor("qT", [P, 8 * QR], bf, kind="ExternalInput").ap()
    kvt = nc.dram_tensor("kvt", [4, 8, P, QR], bf, kind="ExternalInput").ap()
    wq = nc.dram_tensor("wq", [8, P, 8, P], bf, kind="ExternalInput").ap()
    wkv = nc.dram_tensor("wkv", [P, 8, P], bf, kind="ExternalInput").ap()
    wc = nc.dram_tensor("wc", [P, 8 * DM], bf, kind="ExternalInput").ap()
    out = nc.dram_tensor("out", [4, P, DM], bf, kind="ExternalOutput").ap()

    with tile.TileContext(nc) as tc:
        with (
            tc.tile_pool(name="persist", bufs=1) as persist,
            tc.tile_pool(name="kvs", bufs=9) as kvs,
            tc.tile_pool(name="work", bufs=1) as work,
            tc.tile_pool(name="sc_ps", bufs=1, space="PSUM") as sc_ps,
        ):
            kpv = persist.tile([P, L], bf, name="kpv")    # k rows 0:64, v 64:128
            kdup = persist.tile([P, L], bf, name="kdup")  # k dup at rows 64:128
            vT = persist.tile([P, KC, DH], bf, name="vT")
            ones = persist.tile([P, 1], bf, name="ones")
            qpT = persist.tile([P, 8, QR], bf, name="qpT")
            wkv_sb = persist.tile([P, 8, P], bf, name="wkv_sb")
            wq_sb = [
                persist.tile([P, 8, P], bf, name=f"wq_sb{mt}") for mt in range(8)
            ]
            qT_sb = persist.tile([P, 8, QR], bf, name="qT_sb")
            wc_sb = persist.tile([P, 8, DM], bf, name="wc_sb")
            # attn out per qblock: [128q, 4 pairs * 128]; batch a = pairs 0-3
            A2 = [persist.tile([P, 4 * P], bf, name=f"A2_{i}") for i in range(8)]
            outT = [
                persist.tile([P, 4, P], bf, name=f"outT_{i}") for i in range(8)
            ]

            nc.gpsimd.memset(ones, 1.0)

            # ---- input DMAs in priority order ----
            kv_chunks = {}

            def dma_kv_block(nt):
                ch = kvs.tile([P, 8, QR], bf, tag="kv", name="kv_ch", bufs=4)
                nc.sync.dma_start(ch, kvt[nt].rearrange("k p m -> p k m"))
                kv_chunks[nt] = ch

            # DMA order tuned so each PE instruction's inputs land just
            # before the in-order PE dispatch reaches it: q-projection
            # inputs first (PE's first work), all kv blocks before the
            # late-pair wq groups, wc deferred to mid-kernel (its 5.8us
            # transfer must not delay vT transposes on the DMA engines).
            qT3 = qT.rearrange("p (k m) -> p k m", k=8)
            nc.sync.dma_start(wq_sb[0], wq[0])
            # qT in halves: the first q-proj matmuls start after 0.5MB
            nc.sync.dma_start(qT_sb[:, 0:4, :], qT3[:, 0:4, :])
            nc.sync.dma_start(qT_sb[:, 4:8, :], qT3[:, 4:8, :])
            nc.sync.dma_start(wq_sb[1], wq[1])
            nc.sync.dma_start(wq_sb[2], wq[2])
            nc.sync.dma_start(wkv_sb, wkv)
            dma_kv_block(0)
            dma_kv_block(1)
            dma_kv_block(2)
            # kv3 and wq3-7 are emitted inside the pair-0 loop: the DMA
            # engine pool is FIFO by dispatch order, and SP dispatch is
            # in-order, so late bulk loads must enter the queue after the
            # pair-0-critical vT transposes / kdup transfers.

            # ---- kv projection per 512-col block ----
            def kv_block(nt):
                sl = slice(nt * QR, (nt + 1) * QR)
                ps = sc_ps.tile([P, QR], f32, tag="sc", name="ps_kv", bufs=5)
                ch = kv_chunks.pop(nt)
                for kt in range(8):
                    nc.tensor.matmul(
                        ps,
                        wkv_sb[:, kt, :],
                        ch[:, kt, :],
                        start=(kt == 0),
                        stop=(kt == 7),
                    )
                nc.vector.tensor_copy(kpv[:, sl], ps)
                # k dup to rows 64:128 (partition shift via SWDGE)
                nc.gpsimd.dma_start(kdup[DH : 2 * DH, sl], kpv[0:DH, sl])
                # vT[:, 4nt:4nt+4, :] = transpose of v rows (XBAR writes the
                # 256-elem slice contiguously)
                nc.sync.dma_start_transpose(
                    vT[:, 4 * nt : 4 * nt + 4, :], kpv[DH : 2 * DH, sl]
                )

            # ---- q projection: groups 0,1 upfront; 2..7 spread below ----
            qp_ps = {}

            def qp_mm(mt, kt):
                if kt == 0:
                    qp_ps[mt] = sc_ps.tile(
                        [P, QR], f32, tag="qp", name="ps_q", bufs=1
                    )
                nc.tensor.matmul(
                    qp_ps[mt],
                    wq_sb[mt][:, kt, :],
                    qT_sb[:, kt, :],
                    start=(kt == 0),
                    stop=(kt == 7),
                )
                if kt == 7:
                    nc.vector.tensor_copy(qpT[:, mt, :], qp_ps.pop(mt))

            for mt in range(2):
                for kt in range(8):
                    qp_mm(mt, kt)
            kv_block(0)

            # ---- attention ----
            # scores+exp in 2-key-chunk units: [128,1024] tiles amortize the
            # ACT/DVE per-instruction access overhead
            es_q = []  # emitted-but-not-consumed exp tiles: (u, h2, es_bf)

            def do_scores_exp(p, u):
                ksrc = [kpv, kdup]
                for h2 in range(2):
                    lo = h2 * DH
                    for j in range(2):
                        kc = 2 * u + j
                        scp = sc_ps.tile([P, QR], f32, tag="sc", name="scp",
                                         bufs=5)
                        nc.tensor.matmul(
                            scp,
                            ksrc[h2][lo : lo + DH, kc * P : (kc + 1) * P],
                            qpT[lo : lo + DH, p, :],
                            start=True,
                            stop=True,
                        )
                        # engine-split exp: ACT takes even key chunks
                        # exactly, DVE the odd ones via Schraudolph (int16
                        # bits of bf16 exp); both stay under PE's rate
                        if j == 0:
                            es = work.tile([P, QR], bf, tag="es", name="es",
                                           bufs=10)
                            nc.scalar.activation(es, scp, Exp)
                        else:
                            e16 = work.tile([P, QR], i16, tag="es",
                                            name="e16", bufs=10)
                            nc.vector.tensor_scalar(
                                e16, scp, SCHR_A, SCHR_C, Mult, Add
                            )
                            es = e16.bitcast(bf)
                        es_q.append((u, h2, j, es))

            def do_av(av, u, h2, j, es):
                # start=True resets the whole PSUM bank, so only the first
                # group emitted per av tile uses it; the other groups
                # accumulate onto the zeroed bank.
                kc = 2 * u + j
                for qb in range(4):
                    lhsT = es[:, qb * P : (qb + 1) * P]
                    nc.tensor.matmul(
                        av[h2][:, qb, 0:DH],
                        lhsT,
                        vT[:, kc, :],
                        start=(kc == 0 and qb == 0),
                        stop=(kc == KC - 1),
                        skip_group_check=True,
                    )
                    nc.tensor.matmul(
                        av[h2][:, qb, DH : DH + 1],
                        lhsT,
                        ones,
                        start=False,
                        stop=(kc == KC - 1),
                        skip_group_check=True,
                    )

            def drain_av(av, upto):
                # consume queued exp tiles whose unit <= upto
                while es_q and es_q[0][0] <= upto:
                    u, h2, j, es = es_q.pop(0)
                    do_av(av, u, h2, j, es)

            for p in range(8):
                av = [
                    sc_ps.tile([P, 4, DH + 1], f32, tag="av", name="avp",
                               bufs=2)
                    for _ in range(2)
                ]
                for u in range(KC // 2):
                    # ready-first emission: PE dispatch is in-order, so
                    # instructions whose inputs land later go last.
                    # q-proj kt 6+7 both land on u6 so the group's psum
                    # copy clears the single qp bank before the next pair.
                    if p + 2 < 8 and u < 7:
                        qp_mm(p + 2, u)
                        if u == 6:
                            qp_mm(p + 2, 7)
                    if p == 0 and u in (2, 4, 6):
                        kv_block(u // 2)
                        if u == 2:
                            dma_kv_block(3)
                    if p == 0 and u == 5:
                        nc.sync.dma_start(wq_sb[3], wq[3])
                    if p == 0 and u == 7:
                        nc.sync.dma_start(wq_sb[4], wq[4])
                        nc.sync.dma_start(wq_sb[5], wq[5])
                    if p == 1 and u == 0:
                        nc.sync.dma_start(wq_sb[6], wq[6])
                        nc.sync.dma_start(wq_sb[7], wq[7])
                    do_scores_exp(p, u)
                    drain_av(av, u - 1)

                # normalize -> A2; batch a = pairs 0-3, b = 4-7. The copy
                # frees the av PSUM bank fast (emitted right after each
                # h2's last av matmul); gpsimd scales from SBUF.
                base = (p // 4) * 4
                col = (p % 4) * P
                for h2 in range(2):
                    # drain this h2's remaining av work first so avsb can
                    # free its bank for the next pair ASAP
                    while es_q and es_q[0][1] == h2:
                        uu, hh, jj, es = es_q.pop(0)
                        do_av(av, uu, hh, jj, es)
                    avsb = work.tile([P, 4, DH + 1], f32, tag="avsb",
                                     name="avsb", bufs=4)
                    if p >= 5:
                        # late pairs: DVE is the busier engine; ACT copies
                        nc.scalar.activation(avsb, av[h2], Copy)
                    else:
                        nc.vector.tensor_copy(avsb, av[h2])
                    rcp = work.tile([P, 4], f32, tag="rcp", name="rcp", bufs=8)
                    nc.vector.reciprocal(
                        rcp, avsb[:, :, DH]
                    )
                    for qb in range(4):
                        dst = A2[base + qb][:, col + h2 * DH : col + (h2 + 1) * DH]
                        nc.gpsimd.tensor_scalar(
                            dst, avsb[:, qb, 0:DH],
                            rcp[:, qb : qb + 1], None, Mult
                        )
                # eager per-(pair, qb) transpose keeps only pair 7's four
                # transposes on the tail critical path
                for qb in range(4):
                    nc.sync.dma_start_transpose(
                        outT[base + qb][:, p % 4, :],
                        A2[base + qb][:, col : col + P],
                    )
                if p == 3:
                    # wc lands mid-kernel, long before the final matmuls,
                    # without its transfer blocking startup-critical DMAs
                    nc.sync.dma_start(
                        wc_sb, wc.rearrange("p (k m) -> p k m", k=8)
                    )

            # ---- final: out[qb] [128q, 1024] = A @ w_concat ----
            for qb in range(4):
                osb = work.tile([P, DM], bf, tag="osb", name="osb", bufs=2)
                for n in range(2):
                    # alternate between the freed qp bank and the av slots
                    if (2 * qb + n) % 2 == 0:
                        fp = sc_ps.tile([P, QR], f32, name="fp", bufs=1,
                                        tag="qp")
                    else:
                        fp = sc_ps.tile([P, QR], f32, name="fp", bufs=2,
                                        tag="av")
                    for g in range(8):
                        nc.tensor.matmul(
                            fp,
                            outT[(g // 4) * 4 + qb][:, g % 4, :],
                            wc_sb[:, g, n * QR : (n + 1) * QR],
                            start=(g == 0),
                            stop=(g == 7),
                        )
                    if n == 0:
                        nc.scalar.activation(
                            osb[:, n * QR : (n + 1) * QR], fp, Copy
                        )
                    else:
                        nc.vector.tensor_copy(osb[:, n * QR : (n + 1) * QR], fp)
                    nc.sync.dma_start(
                        out[qb][:, n * QR : (n + 1) * QR],
                        osb[:, n * QR : (n + 1) * QR],
                    )

    nc.compile()
    return nc


def _get_nc():
    if "nc" not in _CACHE:
        _CACHE["nc"] = _build_bass()
    return _CACHE["nc"]


def make_in_maps(q, kv, w_q, w_kv, w_concat):
    q = np.asarray(q, np.float32)
    kv = np.asarray(kv, np.float32)
    w_qs = (np.asarray(w_q, np.float32) * 0.125).astype(BF)
    w_kvb = np.asarray(w_kv, np.float32).astype(BF)
    w_cb = np.asarray(w_concat, np.float32).astype(BF)

    # wq[mt, p, kt, m] = w_qs[kt*128+p, mt*128+m]
    wq_t = np.ascontiguousarray(
        w_qs.reshape(8, P, 8, P).transpose(2, 1, 0, 3)
    )
    # wkv[p, kt, m] = w_kv[kt*128+p, m]
    wkv_t = np.ascontiguousarray(w_kvb.reshape(8, P, P).transpose(1, 0, 2))
    # wc[p, kt*1024 + n] = w_concat[kt*128+p, n]
    wc_t = np.ascontiguousarray(
        w_cb.reshape(8, P, DM).transpose(1, 0, 2)
    ).reshape(P, 8 * DM)
    # kvt[nt, kt, p, m] = kv[b].T[kt*128+p, nt*512+m]
    kvt_b = []
    for b in range(B):
        kvT = kv[b].T.astype(BF)  # [1024, 2048]
        kvt_b.append(
            np.ascontiguousarray(
                kvT.reshape(8, P, 4, QR).transpose(2, 0, 1, 3)
            )
        )

    in_maps = []
    for c in range(NCORES):
        b, s = c // 4, (c % 4) * QR
        # qT[p, kt*512+m] = q[b, s+m, kt*128+p]
        qs = q[b, s : s + QR, :].T.astype(BF)  # [1024, 512]
        qT_t = np.ascontiguousarray(
            qs.reshape(8, P, QR).transpose(1, 0, 2)
        ).reshape(P, 8 * QR)
        in_maps.append(
            {
                "qT": qT_t,
                "kvt": kvt_b[b],
                "wq": wq_t,
                "wkv": wkv_t,
                "wc": wc_t,
            }
        )
    return in_maps


def assemble(results):
    full = np.empty((B, L, DM), np.float32)
    for c in range(NCORES):
        b, s = c // 4, (c % 4) * QR
        o = np.asarray(results[c]["out"]).astype(np.float32)  # [4, 128, 1024]
        full[b, s : s + QR, :] = o.reshape(QR, DM)
    return full


def kernel(q, kv, w_q, w_kv, w_concat):
    from concourse.bass_utils import run_bass_kernel_spmd

    nc = _get_nc()
    in_maps = make_in_maps(q, kv, w_q, w_kv, w_concat)
    res = run_bass_kernel_spmd(nc, in_maps, core_ids=list(range(NCORES)))
    return assemble(res.results)
